# revision 1
# baseline (speedup 1.0000x reference)
"""Talking-heads attention (B=4, N=2048, C=384, H=6, d=64) on 8 trn2 cores.

Sharding: data-parallel over (batch b, query-half) -> 8 shards. Each core
computes attention for 1024 query rows of one batch against the full 2048
keys of that batch; tiny weights are replicated.

Algorithmic restructuring (validated exactly vs reference in numpy):
  * pre-softmax talking-heads mix w_l is folded into the Q projection:
      Qbig = x @ Wqbig + bqbig,  Wqbig[c,(g,h,d)] = w_l[h,g]*scale*Wq[c,(h,d)]
    so mixed scores are S[g] = Qbig_g @ K^T (contraction 384, full PE util).
  * key bias b_k and pre-mix bias b_l drop out (softmax row-invariance).
  * scores are tiny (|S| < ~0.1): exp with no max-subtraction; exp'd scores
    E feed PE directly; row sums Z come from a ones-column matmul.
  * post-softmax mix w_w + out-projection fold into one matrix
      Wbig[(g,(g2,d)),c'] = w_w[g,g2]*w_proj[(g2,d),c']
    applied to the per-head cross outputs O[g] = (E_g/Z_g) @ Vcat.
  * V bias + b_w colsum terms fold into a host constant + a per-batch
    device-computed correction row c_row = (b_w ⊙ colsum V) @ w_proj.

Everything on-device runs feature-major (activations transposed), so no
PE transposes are needed anywhere: host supplies x^T, device returns out^T.
"""
import numpy as np
import ml_dtypes

import concourse.bacc as bacc
import concourse.tile as tile
import concourse.mybir as mybir
from concourse.bass_utils import run_bass_kernel_spmd

DIM = 384
HEADS = 6
D = DIM // HEADS
B, N = 4, 2048
M = N
NH = N // 2               # query rows per core
SCALE = D ** -0.5
F32 = mybir.dt.float32
BF16 = mybir.dt.bfloat16
AF = mybir.ActivationFunctionType

_CACHE = {}


def build():
    nc = bacc.Bacc(None, target_bir_lowering=False, debug=False)

    # ---- DRAM parameters (per-core inputs; identical program on all cores)
    d_xt = nc.dram_tensor("xt", [DIM, M], BF16, kind="ExternalInput")
    d_xht = nc.dram_tensor("xht", [DIM, NH], BF16, kind="ExternalInput")
    d_wqbig = nc.dram_tensor("wqbig", [DIM, HEADS * DIM], BF16, kind="ExternalInput")
    d_bqbig = nc.dram_tensor("bqbig", [HEADS * DIM], F32, kind="ExternalInput")
    d_wk = nc.dram_tensor("wk", [DIM, DIM], BF16, kind="ExternalInput")
    d_wv = nc.dram_tensor("wv", [DIM, DIM], BF16, kind="ExternalInput")
    d_wbig = nc.dram_tensor("wbig", [HEADS * DIM, DIM], BF16, kind="ExternalInput")
    d_wproj = nc.dram_tensor("wproj", [DIM, DIM], BF16, kind="ExternalInput")
    d_bwexp = nc.dram_tensor("bwexp", [DIM], F32, kind="ExternalInput")
    d_cbias = nc.dram_tensor("cbias", [DIM], F32, kind="ExternalInput")
    d_out = nc.dram_tensor("out", [DIM, NH], F32, kind="ExternalOutput")

    with tile.TileContext(nc) as tc, \
         tc.tile_pool(name="singles", bufs=1) as singles, \
         tc.tile_pool(name="psA", bufs=2, space="PSUM") as psA, \
         tc.tile_pool(name="psO", bufs=4, space="PSUM") as psO, \
         tc.tile_pool(name="psZ", bufs=1, space="PSUM") as psZ, \
         tc.tile_pool(name="psB", bufs=1, space="PSUM") as psB, \
         tc.tile_pool(name="et_p", bufs=2) as et_p, \
         tc.tile_pool(name="oc_p", bufs=2) as oc_p, \
         tc.tile_pool(name="sm_p", bufs=2) as sm_p, \
         tc.tile_pool(name="out_p", bufs=3) as out_p, \
         tc.tile_pool(name="dram", bufs=1, space="DRAM") as dram:

        # ---- load everything to SBUF (chunked feature-major layouts)
        def load(pool, dparam, shape, rearr, dt, **kw):
            t = pool.tile(shape, dt, name=dparam.name + "_s",
                          tag=dparam.name + "_s")
            nc.sync.dma_start(out=t, in_=dparam.ap().rearrange(rearr, **kw))
            return t

        xt_s = singles.tile([128, 3, M], BF16, name="xt_s", tag="xt_s")
        xht_s = singles.tile([128, 3, NH], BF16, name="xht_s", tag="xht_s")
        wqbig_s = singles.tile([128, 3, HEADS * DIM], BF16, name="wqbig_s",
                               tag="wqbig_s")
        for cc in range(3):
            nc.sync.dma_start(out=xt_s[:, cc, :],
                              in_=d_xt.ap()[cc * 128:(cc + 1) * 128, :])
            nc.sync.dma_start(out=xht_s[:, cc, :],
                              in_=d_xht.ap()[cc * 128:(cc + 1) * 128, :])
            nc.sync.dma_start(out=wqbig_s[:, cc, :],
                              in_=d_wqbig.ap()[cc * 128:(cc + 1) * 128, :])
        bqbig_s = load(singles, d_bqbig, [128, 18], "(fc p) -> p fc", F32, p=128)
        wk_s = load(singles, d_wk, [128, 3, DIM], "(cc p) f -> p cc f", BF16, p=128)
        wv_s = load(singles, d_wv, [128, 3, DIM], "(cc p) f -> p cc f", BF16, p=128)
        wbig_s = load(singles, d_wbig, [128, 18, DIM], "(fc p) c -> p fc c",
                      BF16, p=128)
        wproj_s = load(singles, d_wproj, [128, 3, DIM], "(cc p) f -> p cc f",
                       BF16, p=128)
        bwexp_s = load(singles, d_bwexp, [1, DIM], "(o e) -> o e", F32, o=1)
        cbias_s = load(singles, d_cbias, [128, 3], "(cc p) -> p cc", F32, p=128)

        ones_s = singles.tile([128, 1], BF16)
        nc.vector.memset(ones_s, 1.0)
        onesrow_s = singles.tile([1, 128], BF16)
        nc.vector.memset(onesrow_s, 1.0)

        kt_s = singles.tile([128, 3, M], BF16)        # K^T  [hd, m]
        v_s = singles.tile([128, 16, DIM], BF16)      # V    [m, e]
        qb_s = singles.tile([128, 18, NH], BF16)      # Qbig^T [(g,hd), n]
        fb_s = singles.tile([128, 3], F32)            # c_row + c_bias per c'-chunk

        # ---- prologue: K^T = Wk^T @ x^T
        for fc in range(3):
            for m5 in range(4):
                pt = psA.tile([128, 512], F32, tag="acc")
                for cc in range(3):
                    nc.tensor.matmul(pt, lhsT=wk_s[:, cc, fc * 128:(fc + 1) * 128],
                                     rhs=xt_s[:, cc, m5 * 512:(m5 + 1) * 512],
                                     start=(cc == 0), stop=(cc == 2))
                nc.vector.tensor_copy(out=kt_s[:, fc, m5 * 512:(m5 + 1) * 512],
                                      in_=pt)

        # ---- prologue: V = x @ Wv  (no bias; folded on host)
        for mc in range(16):
            pv = psA.tile([128, 512], F32, tag="acc")
            for cc in range(3):
                nc.tensor.matmul(pv[:, :DIM],
                                 lhsT=xt_s[:, cc, mc * 128:(mc + 1) * 128],
                                 rhs=wv_s[:, cc, :],
                                 start=(cc == 0), stop=(cc == 2))
            nc.vector.tensor_copy(out=v_s[:, mc, :], in_=pv[:, :DIM])

        # ---- S_v = colsum(V);  c_row = (b_w ⊙ S_v) @ w_proj;  fb = c_row+c_bias
        psv = psZ.tile([1, 512], F32, tag="zz")
        for mc in range(16):
            nc.tensor.matmul(psv[:, :DIM], lhsT=ones_s, rhs=v_s[:, mc, :],
                             start=(mc == 0), stop=(mc == 15))
        t_s = sm_p.tile([1, DIM], BF16)
        nc.vector.tensor_mul(out=t_s, in0=psv[:, :DIM], in1=bwexp_s)
        scr = dram.tile([1, DIM], BF16)
        nc.sync.dma_start(out=scr, in_=t_s)
        tT_s = sm_p.tile([128, 3], BF16)
        nc.sync.dma_start(out=tT_s, in_=scr[0].rearrange("(gc p) -> p gc", p=128))
        for ccp in range(3):
            pcr = psB.tile([128, 512], F32, tag="bb")
            for gc in range(3):
                nc.tensor.matmul(pcr[:, :1],
                                 lhsT=wproj_s[:, gc, ccp * 128:(ccp + 1) * 128],
                                 rhs=tT_s[:, gc:gc + 1],
                                 start=(gc == 0), stop=(gc == 2))
            nc.vector.tensor_scalar_add(out=fb_s[:, ccp:ccp + 1], in0=pcr[:, :1],
                                        scalar1=cbias_s[:, ccp:ccp + 1])

        # ---- prologue: Qbig^T = Wqbig^T @ xh^T + bqbig
        for fc in range(18):
            for n5 in range(2):
                pq = psA.tile([128, 512], F32, tag="acc")
                for cc in range(3):
                    nc.tensor.matmul(pq,
                                     lhsT=wqbig_s[:, cc, fc * 128:(fc + 1) * 128],
                                     rhs=xht_s[:, cc, n5 * 512:(n5 + 1) * 512],
                                     start=(cc == 0), stop=(cc == 2))
                nc.vector.tensor_scalar_add(
                    out=qb_s[:, fc, n5 * 512:(n5 + 1) * 512], in0=pq,
                    scalar1=bqbig_s[:, fc:fc + 1])

        # ---- attention: per (n512-chunk, mixed-head g)
        for n5 in range(2):
            ns = slice(n5 * 512, (n5 + 1) * 512)
            ocat = oc_p.tile([128, 18, 512], BF16)
            for g in range(6):
                et = et_p.tile([128, 512, 16], BF16)
                po = [psO.tile([128, 512], F32, tag="po", name=f"po{_ec}")
                      for _ec in range(3)]
                for mc in range(16):
                    ps = psA.tile([128, 512], F32, tag="acc")
                    for c3 in range(3):
                        nc.tensor.matmul(ps,
                                         lhsT=kt_s[:, c3, mc * 128:(mc + 1) * 128],
                                         rhs=qb_s[:, 3 * g + c3, ns],
                                         start=(c3 == 0), stop=(c3 == 2))
                    nc.scalar.activation(out=et[:, :, mc], in_=ps, func=AF.Exp)
                    for ec in range(3):
                        nc.tensor.matmul(po[ec],
                                         lhsT=v_s[:, mc, ec * 128:(ec + 1) * 128],
                                         rhs=et[:, :, mc],
                                         start=(mc == 0), stop=(mc == 15))
                # Z row-sums: partial over m-chunks on DVE, cross-partition on PE
                zsum = sm_p.tile([128, 512], BF16)
                with nc.allow_low_precision(reason="Z partials bf16: ~2e-4 on Z"):
                    nc.vector.reduce_sum(out=zsum, in_=et,
                                         axis=mybir.AxisListType.X)
                pz = psZ.tile([1, 512], F32, tag="zz")
                nc.tensor.matmul(pz, lhsT=ones_s, rhs=zsum, start=True, stop=True)
                rz = sm_p.tile([1, 512], BF16)
                with nc.allow_low_precision(reason="1/Z in bf16: validated 1.5e-3 end-to-end"):
                    nc.vector.reciprocal(out=rz, in_=pz)
                przb = psB.tile([128, 512], F32, tag="bb")
                nc.tensor.matmul(przb, lhsT=onesrow_s, rhs=rz,
                                 start=True, stop=True)
                rzb = sm_p.tile([128, 512], F32)
                nc.scalar.copy(out=rzb, in_=przb)
                for ec in range(3):
                    nc.vector.tensor_mul(out=ocat[:, 3 * g + ec, :],
                                         in0=po[ec], in1=rzb)

            # ---- final projection + bias for this n512 chunk
            for ccp in range(3):
                pf = psA.tile([128, 512], F32, tag="acc")
                for fc in range(18):
                    nc.tensor.matmul(pf,
                                     lhsT=wbig_s[:, fc, ccp * 128:(ccp + 1) * 128],
                                     rhs=ocat[:, fc, :],
                                     start=(fc == 0), stop=(fc == 17))
                ot = out_p.tile([128, 512], F32)
                nc.vector.tensor_scalar_add(out=ot, in0=pf,
                                            scalar1=fb_s[:, ccp:ccp + 1])
                nc.sync.dma_start(
                    out=d_out.ap()[ccp * 128:(ccp + 1) * 128, ns], in_=ot)

    nc.finalize()
    return nc


def _fold(w_qkv, b_qkv, w_l, w_w, b_w, w_proj, b_proj):
    bf = ml_dtypes.bfloat16
    Wq = w_qkv[:, :DIM].reshape(DIM, HEADS, D)
    bq = b_qkv[:DIM].reshape(HEADS, D)
    Wk = w_qkv[:, DIM:2 * DIM]
    Wv = w_qkv[:, 2 * DIM:]
    bv = b_qkv[2 * DIM:].reshape(HEADS, D)

    Wqbig = (np.einsum('chd,hg->cghd', Wq, w_l) * SCALE).reshape(DIM, HEADS * DIM)
    bqbig = (np.einsum('hd,hg->ghd', bq, w_l) * SCALE).reshape(HEADS * DIM)
    w_proj_r = w_proj.reshape(HEADS, D, DIM)
    Wbig = np.einsum('gz,zdc->gzdc', w_w, w_proj_r).reshape(HEADS * DIM, DIM)
    c_bias = (b_proj
              + np.einsum('gz,zdc,zd->c', w_w, w_proj_r, bv)
              + M * np.einsum('z,zdc,zd->c', b_w, w_proj_r, bv))
    bwexp = np.repeat(b_w, D)
    return dict(wqbig=Wqbig.astype(bf), bqbig=bqbig.astype(np.float32),
                wk=Wk.astype(bf), wv=Wv.astype(bf), wbig=Wbig.astype(bf),
                wproj=w_proj.astype(bf), bwexp=bwexp.astype(np.float32),
                cbias=c_bias.astype(np.float32))


def kernel(**inputs):
    x = np.asarray(inputs["x"], np.float32)
    f = _fold(*[np.asarray(inputs[k], np.float32) for k in
                ("w_qkv", "b_qkv", "w_l", "w_w", "b_w", "w_proj", "b_proj")])

    if "nc" not in _CACHE:
        _CACHE["nc"] = build()
    nc = _CACHE["nc"]

    bf = ml_dtypes.bfloat16
    in_maps = []
    for core in range(8):
        b, half = core // 2, core % 2
        xT = np.ascontiguousarray(x[b].T).astype(bf)
        in_maps.append({
            "xt": xT,
            "xht": np.ascontiguousarray(xT[:, half * NH:(half + 1) * NH]),
            **f,
        })
    import os
    trace = bool(int(os.environ.get("BASSK_TRACE", "0")))
    res = run_bass_kernel_spmd(nc, in_maps, core_ids=list(range(8)),
                               trace=trace)
    _CACHE["last_results"] = res

    out = np.empty((B, N, DIM), np.float32)
    for core in range(8):
        b, half = core // 2, core % 2
        out[b, half * NH:(half + 1) * NH, :] = res.results[core]["out"].T
    return out



# revision 2
# speedup vs baseline: 2.9564x; 2.9564x over previous
"""Talking-heads attention (B=4, N=2048, C=384, H=6, d=64) on 8 trn2 cores.

Sharding: data-parallel over (batch b, query-half) -> 8 shards. Each core
computes attention for 1024 query rows of one batch against the full 2048
keys of that batch; tiny weights are replicated.

Algorithmic restructuring (validated exactly vs reference in numpy):
  * pre-softmax talking-heads mix w_l is folded into the Q projection:
      Qbig = x @ Wqbig + bqbig,  Wqbig[c,(g,h,d)] = w_l[h,g]*scale*Wq[c,(h,d)]
    so mixed scores are S[g] = Qbig_g @ K^T (contraction 384, full PE util).
  * key bias b_k and pre-mix bias b_l drop out (softmax row-invariance).
  * scores are tiny (|S| < ~0.1): exp with no max-subtraction.
  * post-softmax mix w_w + out-projection fold into one matrix
      Wbig[(g,(g2,d)),c'] = w_w[g,g2]*w_proj[(g2,d),c']
    applied to the per-head cross outputs O[g] = (E_g/Z_g) @ Vcat.
  * V bias + b_w colsum terms fold into a host constant + a per-batch
    device-computed correction row c_row = (b_w ⊙ colsum V) @ w_proj.

fp8 acceleration: the two dominant GEMMs (scores, A@V — 9.7 GFLOP each per
core) run in fp8e4 with perf_mode=DoubleRow (256-deep contraction per
instruction). Scales are folded into host weights: K,V are x16, Qbig x2048,
so fp8 operands sit in e4m3's sweet range. E is centered (E-1)*16 before
quantization so the attention weights' fluctuation survives fp8; the
removed DC term Σ_m V[m,:] is restored exactly in PSUM from a bf16-V
column-sum, which also cancels V's fp8 quantization error on the output's
DC component (validated: rel_l2 1.55e-3, same as the all-bf16 version).

Everything on-device runs feature-major (activations transposed), so no
PE transposes are needed anywhere: host supplies x^T, device returns out^T.
"""
import numpy as np
import ml_dtypes

import concourse.bacc as bacc
import concourse.tile as tile
import concourse.mybir as mybir
from concourse.bass_utils import run_bass_kernel_spmd

DIM = 384
HEADS = 6
D = DIM // HEADS
B, N = 4, 2048
M = N
NH = N // 2               # query rows per core
SCALE = D ** -0.5
F32 = mybir.dt.float32
BF16 = mybir.dt.bfloat16
FP8 = mybir.dt.float8e4
AF = mybir.ActivationFunctionType
ALU = mybir.AluOpType
DR = mybir.MatmulPerfMode.DoubleRow

AK = 16.0                 # fp8 scale on K   (folded into w_k on host)
AQ = 2048.0               # fp8 scale on Qbig (folded into w_qbig on host)
AV = 16.0                 # fp8 scale on V   (folded into w_v on host)
SE = 16.0                 # fp8 scale on (E - 1)

_CACHE = {}


def build():
    nc = bacc.Bacc(None, target_bir_lowering=False, debug=False)

    # ---- DRAM parameters (per-core inputs; identical program on all cores)
    d_xt = nc.dram_tensor("xt", [DIM, M], BF16, kind="ExternalInput")
    d_xht = nc.dram_tensor("xht", [DIM, NH], BF16, kind="ExternalInput")
    d_wqbig = nc.dram_tensor("wqbig", [DIM, HEADS * DIM], BF16, kind="ExternalInput")
    d_bqbig = nc.dram_tensor("bqbig", [HEADS * DIM], F32, kind="ExternalInput")
    d_wk = nc.dram_tensor("wk", [DIM, DIM], BF16, kind="ExternalInput")
    d_wv = nc.dram_tensor("wv", [DIM, DIM], BF16, kind="ExternalInput")
    d_wbig = nc.dram_tensor("wbig", [HEADS * DIM, DIM], BF16, kind="ExternalInput")
    d_wproj = nc.dram_tensor("wproj", [DIM, DIM], BF16, kind="ExternalInput")
    d_bwexp = nc.dram_tensor("bwexp", [DIM], F32, kind="ExternalInput")
    d_cbias = nc.dram_tensor("cbias", [DIM], F32, kind="ExternalInput")
    d_out = nc.dram_tensor("out", [DIM, NH], F32, kind="ExternalOutput")

    with tile.TileContext(nc) as tc, \
         tc.tile_pool(name="singles", bufs=1) as singles, \
         tc.tile_pool(name="psA", bufs=2, space="PSUM") as psA, \
         tc.tile_pool(name="psO", bufs=4, space="PSUM") as psO, \
         tc.tile_pool(name="psZ", bufs=1, space="PSUM") as psZ, \
         tc.tile_pool(name="psB", bufs=1, space="PSUM") as psB, \
         tc.tile_pool(name="et_p", bufs=2) as et_p, \
         tc.tile_pool(name="es_p", bufs=3) as es_p, \
         tc.tile_pool(name="z_p", bufs=2) as z_p, \
         tc.tile_pool(name="oc_p", bufs=2) as oc_p, \
         tc.tile_pool(name="sm_p", bufs=2) as sm_p, \
         tc.tile_pool(name="out_p", bufs=3) as out_p, \
         tc.tile_pool(name="dram", bufs=1, space="DRAM") as dram:

        # ---- load everything to SBUF (chunked feature-major layouts)
        def load(pool, dparam, shape, rearr, dt, **kw):
            t = pool.tile(shape, dt, name=dparam.name + "_s",
                          tag=dparam.name + "_s")
            nc.sync.dma_start(out=t, in_=dparam.ap().rearrange(rearr, **kw))
            return t

        xt_s = singles.tile([128, 3, M], BF16, name="xt_s", tag="xt_s")
        xht_s = singles.tile([128, 3, NH], BF16, name="xht_s", tag="xht_s")
        wqbig_s = singles.tile([128, 3, HEADS * DIM], BF16, name="wqbig_s",
                               tag="wqbig_s")
        for cc in range(3):
            nc.sync.dma_start(out=xt_s[:, cc, :],
                              in_=d_xt.ap()[cc * 128:(cc + 1) * 128, :])
            nc.sync.dma_start(out=xht_s[:, cc, :],
                              in_=d_xht.ap()[cc * 128:(cc + 1) * 128, :])
            nc.sync.dma_start(out=wqbig_s[:, cc, :],
                              in_=d_wqbig.ap()[cc * 128:(cc + 1) * 128, :])
        bqbig_s = load(singles, d_bqbig, [128, 18], "(fc p) -> p fc", F32, p=128)
        wk_s = load(singles, d_wk, [128, 3, DIM], "(cc p) f -> p cc f", BF16, p=128)
        wv_s = load(singles, d_wv, [128, 3, DIM], "(cc p) f -> p cc f", BF16, p=128)
        wbig_s = load(singles, d_wbig, [128, 18, DIM], "(fc p) c -> p fc c",
                      BF16, p=128)
        wproj_s = load(singles, d_wproj, [128, 3, DIM], "(cc p) f -> p cc f",
                       BF16, p=128)
        bwexp_s = load(singles, d_bwexp, [1, DIM], "(o e) -> o e", F32, o=1)
        cbias_s = load(singles, d_cbias, [128, 3], "(cc p) -> p cc", F32, p=128)

        ones_s = singles.tile([128, 1], BF16)
        nc.vector.memset(ones_s, 1.0)
        onesrow_s = singles.tile([1, 128], BF16)
        nc.vector.memset(onesrow_s, 1.0)

        kt8_s = singles.tile([128, 3, M], FP8)        # fp8 K^T  [hd, m], x16
        v_s = singles.tile([128, 16, DIM], BF16)      # bf16 V (colsum path), x16
        v8_s = singles.tile([128, 16, DIM], FP8)      # fp8 V, x16
        qb8_s = singles.tile([128, 18, NH], FP8)      # fp8 Qbig^T [(g,hd), n], x2048
        fb_s = singles.tile([128, 3], F32)            # c_row + c_bias per c'-chunk
        cv_s = singles.tile([128, 3], F32)            # 256*colsumV per c'-chunk

        # ---- prologue: K^T = Wk^T @ x^T  (x16), quantize to fp8
        for fc in range(3):
            for m5 in range(4):
                pt = psA.tile([128, 512], F32, tag="acc")
                for cc in range(3):
                    nc.tensor.matmul(pt, lhsT=wk_s[:, cc, fc * 128:(fc + 1) * 128],
                                     rhs=xt_s[:, cc, m5 * 512:(m5 + 1) * 512],
                                     start=(cc == 0), stop=(cc == 2))
                with nc.allow_low_precision(reason="K quantized to fp8e4 (x16 scale); validated 1.6e-3 end-to-end"):
                    nc.vector.tensor_copy(out=kt8_s[:, fc, m5 * 512:(m5 + 1) * 512],
                                          in_=pt)

        # ---- prologue: V = x @ Wv (x16); bf16 copy for colsum + fp8 for AV
        for mc in range(16):
            pv = psA.tile([128, 512], F32, tag="acc")
            for cc in range(3):
                nc.tensor.matmul(pv[:, :DIM],
                                 lhsT=xt_s[:, cc, mc * 128:(mc + 1) * 128],
                                 rhs=wv_s[:, cc, :],
                                 start=(cc == 0), stop=(cc == 2))
            nc.scalar.copy(out=v_s[:, mc, :], in_=pv[:, :DIM])
            with nc.allow_low_precision(reason="V quantized to fp8e4 (x16 scale); DC error cancelled via bf16 colsum"):
                nc.vector.tensor_copy(out=v8_s[:, mc, :], in_=pv[:, :DIM])

        # ---- S_v = colsum(V)*16;  c_row = (b_w ⊙ colsumV) @ w_proj;  fb, cv
        psv = psZ.tile([1, 512], F32, tag="zz")
        for mc in range(16):
            nc.tensor.matmul(psv[:, :DIM], lhsT=ones_s, rhs=v_s[:, mc, :],
                             start=(mc == 0), stop=(mc == 15))
        t_s = sm_p.tile([1, DIM], BF16)
        nc.vector.tensor_mul(out=t_s, in0=psv[:, :DIM], in1=bwexp_s)
        u_s = sm_p.tile([1, DIM], F32, name="u_s", tag="u_s")
        nc.vector.tensor_scalar_mul(out=u_s, in0=psv[:, :DIM], scalar1=SE)
        scr = dram.tile([1, DIM], BF16)
        nc.sync.dma_start(out=scr, in_=t_s)
        scr2 = dram.tile([1, DIM], F32, name="scr2", tag="scr2")
        nc.sync.dma_start(out=scr2, in_=u_s)
        tT_s = sm_p.tile([128, 3], BF16)
        nc.sync.dma_start(out=tT_s, in_=scr[0].rearrange("(gc p) -> p gc", p=128))
        nc.sync.dma_start(out=cv_s, in_=scr2[0].rearrange("(gc p) -> p gc", p=128))
        for ccp in range(3):
            pcr = psB.tile([128, 512], F32, tag="bb")
            for gc in range(3):
                nc.tensor.matmul(pcr[:, :1],
                                 lhsT=wproj_s[:, gc, ccp * 128:(ccp + 1) * 128],
                                 rhs=tT_s[:, gc:gc + 1],
                                 start=(gc == 0), stop=(gc == 2))
            nc.vector.tensor_scalar_add(out=fb_s[:, ccp:ccp + 1], in0=pcr[:, :1],
                                        scalar1=cbias_s[:, ccp:ccp + 1])

        # ---- prologue: Qbig^T = Wqbig^T @ xh^T + bqbig (x2048), quantize fp8
        for fc in range(18):
            for n5 in range(2):
                pq = psA.tile([128, 512], F32, tag="acc")
                for cc in range(3):
                    nc.tensor.matmul(pq,
                                     lhsT=wqbig_s[:, cc, fc * 128:(fc + 1) * 128],
                                     rhs=xht_s[:, cc, n5 * 512:(n5 + 1) * 512],
                                     start=(cc == 0), stop=(cc == 2))
                with nc.allow_low_precision(reason="Qbig quantized to fp8e4 (x2048 scale); validated 1.6e-3 end-to-end"):
                    nc.vector.tensor_scalar_add(
                        out=qb8_s[:, fc, n5 * 512:(n5 + 1) * 512], in0=pq,
                        scalar1=bqbig_s[:, fc:fc + 1])

        # ---- attention: per (n512-chunk, mixed-head g)
        for n5 in range(2):
            ns = slice(n5 * 512, (n5 + 1) * 512)
            ocat = oc_p.tile([128, 18, 512], BF16)
            for g in range(6):
                et = et_p.tile([128, 16, 512], FP8)   # (E-1)*16 fp8, chunk-major
                zacc = z_p.tile([128, 512], BF16)     # per-chunk partial Z sums
                po = [psO.tile([128, 512], F32, tag="po", name=f"po{_ec}")
                      for _ec in range(3)]
                for mc in range(16):
                    ps = psA.tile([128, 512], F32, tag="acc")
                    # scores: fp8 DoubleRow over c-chunks {0,1}, plain fp8 chunk 2
                    nc.tensor.matmul(ps,
                                     lhsT=kt8_s[:, 0:2, mc * 128:(mc + 1) * 128],
                                     rhs=qb8_s[:, 3 * g:3 * g + 2, ns],
                                     start=True, stop=False, perf_mode=DR)
                    nc.tensor.matmul(ps,
                                     lhsT=kt8_s[:, 2, mc * 128:(mc + 1) * 128],
                                     rhs=qb8_s[:, 3 * g + 2, ns],
                                     start=False, stop=True)
                    es = es_p.tile([128, 512], BF16)
                    nc.scalar.activation(out=es, in_=ps, func=AF.Exp,
                                         scale=1.0 / (AK * AQ))
                    with nc.allow_low_precision(reason="(E-1)*16 in fp8e4 + bf16 Z partials: validated 1.6e-3 end-to-end"):
                        nc.vector.tensor_scalar(out=et[:, mc, :], in0=es,
                                                scalar1=1.0, scalar2=SE,
                                                op0=ALU.subtract, op1=ALU.mult)
                        if mc == 0:
                            nc.vector.tensor_copy(out=zacc, in_=es)
                        else:
                            nc.vector.tensor_add(out=zacc, in0=zacc, in1=es)
                    if mc % 2 == 1:
                        j = mc // 2
                        for ec in range(3):
                            nc.tensor.matmul(
                                po[ec],
                                lhsT=v8_s[:, 2 * j:2 * j + 2,
                                          ec * 128:(ec + 1) * 128],
                                rhs=et[:, 2 * j:2 * j + 2, :],
                                start=(j == 0), stop=(j == 7), perf_mode=DR)
                # Z row-sums: cross-partition via PE; rzs = 1/(256*Z)
                pz = psZ.tile([1, 512], F32, tag="zz")
                nc.tensor.matmul(pz, lhsT=ones_s, rhs=zacc, start=True, stop=True)
                tz = sm_p.tile([1, 512], F32, name="tz", tag="tz")
                nc.vector.tensor_scalar_mul(out=tz, in0=pz, scalar1=AV * SE)
                rz = sm_p.tile([1, 512], BF16)
                with nc.allow_low_precision(reason="1/Z in bf16: validated 1.5e-3 end-to-end"):
                    nc.vector.reciprocal(out=rz, in_=tz)
                przb = psB.tile([128, 512], F32, tag="bb")
                nc.tensor.matmul(przb, lhsT=onesrow_s, rhs=rz,
                                 start=True, stop=True)
                rzb = sm_p.tile([128, 512], F32)
                nc.scalar.copy(out=rzb, in_=przb)
                with nc.allow_low_precision(reason="ocat bf16: validated 1.5e-3 end-to-end"):
                    for ec in range(3):
                        nc.vector.scalar_tensor_tensor(
                            out=ocat[:, 3 * g + ec, :], in0=po[ec],
                            scalar=cv_s[:, ec:ec + 1], in1=rzb,
                            op0=ALU.add, op1=ALU.mult)

            # ---- final projection + bias for this n512 chunk
            for ccp in range(3):
                pf = psA.tile([128, 512], F32, tag="acc")
                for fc in range(18):
                    nc.tensor.matmul(pf,
                                     lhsT=wbig_s[:, fc, ccp * 128:(ccp + 1) * 128],
                                     rhs=ocat[:, fc, :],
                                     start=(fc == 0), stop=(fc == 17))
                ot = out_p.tile([128, 512], F32)
                nc.vector.tensor_scalar_add(out=ot, in0=pf,
                                            scalar1=fb_s[:, ccp:ccp + 1])
                nc.sync.dma_start(
                    out=d_out.ap()[ccp * 128:(ccp + 1) * 128, ns], in_=ot)

    nc.finalize()
    return nc


def _fold(w_qkv, b_qkv, w_l, w_w, b_w, w_proj, b_proj):
    bf = ml_dtypes.bfloat16
    Wq = w_qkv[:, :DIM].reshape(DIM, HEADS, D)
    bq = b_qkv[:DIM].reshape(HEADS, D)
    Wk = w_qkv[:, DIM:2 * DIM]
    Wv = w_qkv[:, 2 * DIM:]
    bv = b_qkv[2 * DIM:].reshape(HEADS, D)

    Wqbig = (np.einsum('chd,hg->cghd', Wq, w_l) * SCALE).reshape(DIM, HEADS * DIM)
    bqbig = (np.einsum('hd,hg->ghd', bq, w_l) * SCALE).reshape(HEADS * DIM)
    w_proj_r = w_proj.reshape(HEADS, D, DIM)
    Wbig = np.einsum('gz,zdc->gzdc', w_w, w_proj_r).reshape(HEADS * DIM, DIM)
    c_bias = (b_proj
              + np.einsum('gz,zdc,zd->c', w_w, w_proj_r, bv)
              + M * np.einsum('z,zdc,zd->c', b_w, w_proj_r, bv))
    bwexp = np.repeat(b_w, D) / AV
    return dict(wqbig=(Wqbig * AQ).astype(bf), bqbig=(bqbig * AQ).astype(np.float32),
                wk=(Wk * AK).astype(bf), wv=(Wv * AV).astype(bf),
                wbig=Wbig.astype(bf),
                wproj=w_proj.astype(bf), bwexp=bwexp.astype(np.float32),
                cbias=c_bias.astype(np.float32))


def kernel(**inputs):
    x = np.asarray(inputs["x"], np.float32)
    f = _fold(*[np.asarray(inputs[k], np.float32) for k in
                ("w_qkv", "b_qkv", "w_l", "w_w", "b_w", "w_proj", "b_proj")])

    if "nc" not in _CACHE:
        _CACHE["nc"] = build()
    nc = _CACHE["nc"]

    bf = ml_dtypes.bfloat16
    in_maps = []
    for core in range(8):
        b, half = core // 2, core % 2
        xT = np.ascontiguousarray(x[b].T).astype(bf)
        in_maps.append({
            "xt": xT,
            "xht": np.ascontiguousarray(xT[:, half * NH:(half + 1) * NH]),
            **f,
        })
    import os
    trace = bool(int(os.environ.get("BASSK_TRACE", "0")))
    res = run_bass_kernel_spmd(nc, in_maps, core_ids=list(range(8)),
                               trace=trace)
    _CACHE["last_results"] = res

    out = np.empty((B, N, DIM), np.float32)
    for core in range(8):
        b, half = core // 2, core % 2
        out[b, half * NH:(half + 1) * NH, :] = res.results[core]["out"].T
    return out


# revision 6
# speedup vs baseline: 3.2162x; 1.0879x over previous
"""Talking-heads attention (B=4, N=2048, C=384, H=6, d=64) on 8 trn2 cores.

Sharding: data-parallel over (batch b, query-half) -> 8 shards. Each core
computes attention for 1024 query rows of one batch against the full 2048
keys of that batch; tiny weights are replicated.

Algorithmic restructuring (validated exactly vs reference in numpy):
  * pre-softmax talking-heads mix w_l is folded into the Q projection:
      Qbig = x @ Wqbig + bqbig,  Wqbig[c,(g,h,d)] = w_l[h,g]*scale*Wq[c,(h,d)]
    so mixed scores are S[g] = Qbig_g @ K^T (contraction 384, full PE util).
  * key bias b_k and pre-mix bias b_l drop out (softmax row-invariance).
  * scores are tiny (|S| < ~0.1): exp with no max-subtraction.
  * post-softmax mix w_w + out-projection fold into one matrix
      Wbig[(g,(g2,d)),c'] = w_w[g,g2]*w_proj[(g2,d),c']
    applied to the per-head cross outputs O[g] = (E_g/Z_g) @ Vcat.
  * V bias + b_w colsum terms fold into a host constant + a per-batch
    device-computed correction row c_row = (b_w ⊙ colsum V) @ w_proj.

fp8 acceleration: the two dominant GEMMs (scores, A@V — 9.7 GFLOP each per
core) run in fp8e4 with perf_mode=DoubleRow (256-deep contraction per
instruction). Scales are folded into host weights: K,V are x16, Qbig x2048,
so fp8 operands sit in e4m3's sweet range. E is centered (E-1)*16 before
quantization so the attention weights' fluctuation survives fp8; the
removed DC term Σ_m V[m,:] is restored exactly in PSUM from a bf16-V
column-sum, which also cancels V's fp8 quantization error on the output's
DC component (validated: rel_l2 1.55e-3, same as the all-bf16 version).

Everything on-device runs feature-major (activations transposed), so no
PE transposes are needed anywhere: host supplies x^T, device returns out^T.
"""
import numpy as np
import ml_dtypes

import concourse.bacc as bacc
import concourse.tile as tile
import concourse.mybir as mybir
from concourse.bass_utils import run_bass_kernel_spmd

DIM = 384
HEADS = 6
D = DIM // HEADS
B, N = 4, 2048
M = N
NH = N // 2               # query rows per core
SCALE = D ** -0.5
F32 = mybir.dt.float32
BF16 = mybir.dt.bfloat16
FP8 = mybir.dt.float8e4
AF = mybir.ActivationFunctionType
ALU = mybir.AluOpType
DR = mybir.MatmulPerfMode.DoubleRow

AK = 16.0                 # fp8 scale on K   (folded into w_k on host)
AQ = 2048.0               # fp8 scale on Qbig (folded into w_qbig on host)
AV = 16.0                 # fp8 scale on V   (folded into w_v on host)
SE = 16.0                 # fp8 scale on (E - 1)

_CACHE = {}


def build():
    nc = bacc.Bacc(None, target_bir_lowering=False, debug=False)

    # ---- DRAM parameters (per-core inputs; identical program on all cores)
    d_xt = nc.dram_tensor("xt", [DIM, M], BF16, kind="ExternalInput")
    d_xht = nc.dram_tensor("xht", [DIM, NH], BF16, kind="ExternalInput")
    d_wqbig = nc.dram_tensor("wqbig", [DIM, HEADS * DIM], BF16, kind="ExternalInput")
    d_bqbig = nc.dram_tensor("bqbig", [HEADS * DIM], F32, kind="ExternalInput")
    d_wk = nc.dram_tensor("wk", [DIM, DIM], BF16, kind="ExternalInput")
    d_wv = nc.dram_tensor("wv", [DIM, DIM], BF16, kind="ExternalInput")
    d_wbig = nc.dram_tensor("wbig", [HEADS * DIM, DIM], BF16, kind="ExternalInput")
    d_wproj = nc.dram_tensor("wproj", [DIM, DIM], BF16, kind="ExternalInput")
    d_bwexp = nc.dram_tensor("bwexp", [DIM], F32, kind="ExternalInput")
    d_cbias = nc.dram_tensor("cbias", [DIM], F32, kind="ExternalInput")
    d_out = nc.dram_tensor("out", [DIM, NH], F32, kind="ExternalOutput")

    with tile.TileContext(nc) as tc, \
         tc.tile_pool(name="singles", bufs=1) as singles, \
         tc.tile_pool(name="psA", bufs=2, space="PSUM") as psA, \
         tc.tile_pool(name="psO", bufs=3, space="PSUM") as psO, \
         tc.tile_pool(name="psB", bufs=1, space="PSUM") as psB, \
         tc.tile_pool(name="et_p", bufs=2) as et_p, \
         tc.tile_pool(name="es_p", bufs=3) as es_p, \
         tc.tile_pool(name="z_p", bufs=2) as z_p, \
         tc.tile_pool(name="oc_p", bufs=2) as oc_p, \
         tc.tile_pool(name="sm_p", bufs=2) as sm_p, \
         tc.tile_pool(name="out_p", bufs=3) as out_p, \
         tc.tile_pool(name="dram", bufs=1, space="DRAM") as dram:

        # ---- load everything to SBUF (chunked feature-major layouts)
        def load(pool, dparam, shape, rearr, dt, **kw):
            t = pool.tile(shape, dt, name=dparam.name + "_s",
                          tag=dparam.name + "_s")
            nc.sync.dma_start(out=t, in_=dparam.ap().rearrange(rearr, **kw))
            return t

        xt_s = singles.tile([128, 3, M], BF16, name="xt_s", tag="xt_s")
        xht_s = singles.tile([128, 3, NH], BF16, name="xht_s", tag="xht_s")
        wqbig_s = singles.tile([128, 3, HEADS * DIM], BF16, name="wqbig_s",
                               tag="wqbig_s")
        for cc in range(3):
            nc.sync.dma_start(out=xt_s[:, cc, :],
                              in_=d_xt.ap()[cc * 128:(cc + 1) * 128, :])
            nc.sync.dma_start(out=xht_s[:, cc, :],
                              in_=d_xht.ap()[cc * 128:(cc + 1) * 128, :])
            nc.sync.dma_start(out=wqbig_s[:, cc, :],
                              in_=d_wqbig.ap()[cc * 128:(cc + 1) * 128, :])
        bqbig_s = load(singles, d_bqbig, [128, 18], "(fc p) -> p fc", F32, p=128)
        wk_s = load(singles, d_wk, [128, 3, DIM], "(cc p) f -> p cc f", BF16, p=128)
        wv_s = load(singles, d_wv, [128, 3, DIM], "(cc p) f -> p cc f", BF16, p=128)
        wbig_s = load(singles, d_wbig, [128, 18, DIM], "(fc p) c -> p fc c",
                      BF16, p=128)
        wproj_s = load(singles, d_wproj, [128, 3, DIM], "(cc p) f -> p cc f",
                       BF16, p=128)
        bwexp_s = load(singles, d_bwexp, [1, DIM], "(o e) -> o e", F32, o=1)
        cbias_s = load(singles, d_cbias, [128, 3], "(cc p) -> p cc", F32, p=128)

        ones_s = singles.tile([128, 1], BF16)
        nc.vector.memset(ones_s, 1.0)
        onesrow_s = singles.tile([1, 128], BF16)
        nc.vector.memset(onesrow_s, 1.0)

        kt8_s = singles.tile([128, 3, M], FP8)        # fp8 K^T  [hd, m], x16
        v_s = singles.tile([128, 16, DIM], BF16)      # bf16 V (colsum path), x16
        v8_s = singles.tile([128, 16, DIM], FP8)      # fp8 V, x16
        qb8_s = singles.tile([128, 18, NH], FP8)      # fp8 Qbig^T [(g,hd), n], x2048
        fb_s = singles.tile([128, 3], F32)            # c_row + c_bias per c'-chunk
        cv_s = singles.tile([128, 3], F32)            # 256*colsumV per c'-chunk

        # ---- prologue: K^T = Wk^T @ x^T  (x16), quantize to fp8
        for fc in range(3):
            for m5 in range(4):
                pt = psA.tile([128, 2, 512], F32, tag="acc")
                for cc in range(3):
                    nc.tensor.matmul(pt[:, 0, :],
                                     lhsT=wk_s[:, cc, fc * 128:(fc + 1) * 128],
                                     rhs=xt_s[:, cc, m5 * 512:(m5 + 1) * 512],
                                     start=(cc == 0), stop=(cc == 2))
                with nc.allow_low_precision(reason="K quantized to fp8e4 (x16 scale); validated 1.6e-3 end-to-end"):
                    nc.vector.tensor_copy(out=kt8_s[:, fc, m5 * 512:(m5 + 1) * 512],
                                          in_=pt[:, 0, :])

        # ---- prologue: V = x @ Wv (x16); bf16 copy for colsum + fp8 for AV
        for mc in range(16):
            pv = psA.tile([128, 2, 512], F32, tag="acc")
            for cc in range(3):
                nc.tensor.matmul(pv[:, 0, :DIM],
                                 lhsT=xt_s[:, cc, mc * 128:(mc + 1) * 128],
                                 rhs=wv_s[:, cc, :],
                                 start=(cc == 0), stop=(cc == 2))
            nc.scalar.copy(out=v_s[:, mc, :], in_=pv[:, 0, :DIM])
            with nc.allow_low_precision(reason="V quantized to fp8e4 (x16 scale); DC error cancelled via bf16 colsum"):
                nc.vector.tensor_copy(out=v8_s[:, mc, :], in_=pv[:, 0, :DIM])

        # ---- S_v = colsum(V)*16;  c_row = (b_w ⊙ colsumV) @ w_proj;  fb, cv
        psv = psB.tile([1, 512], F32, tag="bb", name="psv")
        for mc in range(16):
            nc.tensor.matmul(psv[:, :DIM], lhsT=ones_s, rhs=v_s[:, mc, :],
                             start=(mc == 0), stop=(mc == 15))
        t_s = sm_p.tile([1, DIM], BF16)
        nc.vector.tensor_mul(out=t_s, in0=psv[:, :DIM], in1=bwexp_s)
        u_s = sm_p.tile([1, DIM], F32, name="u_s", tag="u_s")
        nc.vector.tensor_scalar_mul(out=u_s, in0=psv[:, :DIM], scalar1=SE)
        scr = dram.tile([1, DIM], BF16)
        nc.sync.dma_start(out=scr, in_=t_s)
        scr2 = dram.tile([1, DIM], F32, name="scr2", tag="scr2")
        nc.sync.dma_start(out=scr2, in_=u_s)
        tT_s = sm_p.tile([128, 3], BF16)
        nc.sync.dma_start(out=tT_s, in_=scr[0].rearrange("(gc p) -> p gc", p=128))
        nc.sync.dma_start(out=cv_s, in_=scr2[0].rearrange("(gc p) -> p gc", p=128))
        for ccp in range(3):
            pcr = psB.tile([128, 512], F32, tag="bb")
            for gc in range(3):
                nc.tensor.matmul(pcr[:, :1],
                                 lhsT=wproj_s[:, gc, ccp * 128:(ccp + 1) * 128],
                                 rhs=tT_s[:, gc:gc + 1],
                                 start=(gc == 0), stop=(gc == 2))
            nc.vector.tensor_scalar_add(out=fb_s[:, ccp:ccp + 1], in0=pcr[:, :1],
                                        scalar1=cbias_s[:, ccp:ccp + 1])

        # ---- prologue: Qbig^T = Wqbig^T @ xh^T + bqbig (x2048), quantize fp8
        for fc in range(18):
            for n5 in range(2):
                pq = psA.tile([128, 2, 512], F32, tag="acc")
                for cc in range(3):
                    nc.tensor.matmul(pq[:, 0, :],
                                     lhsT=wqbig_s[:, cc, fc * 128:(fc + 1) * 128],
                                     rhs=xht_s[:, cc, n5 * 512:(n5 + 1) * 512],
                                     start=(cc == 0), stop=(cc == 2))
                with nc.allow_low_precision(reason="Qbig quantized to fp8e4 (x2048 scale); validated 1.6e-3 end-to-end"):
                    nc.scalar.activation(
                        out=qb8_s[:, fc, n5 * 512:(n5 + 1) * 512], in_=pq[:, 0, :],
                        func=AF.Identity, bias=bqbig_s[:, fc:fc + 1])

        # ---- attention: per (n512-chunk, mixed-head g)
        onesb_s = singles.tile([128, 128], BF16)
        nc.vector.memset(onesb_s, 1.0)
        negln_s = singles.tile([128, 1], F32)
        nc.vector.memset(negln_s, -float(np.log(AV * SE)))
        for n5 in range(2):
            ns = slice(n5 * 512, (n5 + 1) * 512)
            ocat = oc_p.tile([128, 18, 512], BF16)
            for g in range(6):
                et = et_p.tile([128, 16, 512], FP8)   # (E-1)*16 fp8, chunk-major
                zacc = z_p.tile([128, 2, 512], BF16)  # paired partial Z sums
                po = [psO.tile([128, 512], F32, tag="po", name=f"po{_ec}")
                      for _ec in range(3)]
                for j in range(8):                    # pairs of 128-key chunks
                    ps = psA.tile([128, 2, 512], F32, tag="acc")
                    for jj in range(2):
                        mc = 2 * j + jj
                        # scores: fp8 DoubleRow c-chunks {0,1}, plain fp8 chunk 2
                        nc.tensor.matmul(ps[:, jj, :],
                                         lhsT=kt8_s[:, 0:2, mc * 128:(mc + 1) * 128],
                                         rhs=qb8_s[:, 3 * g:3 * g + 2, ns],
                                         start=True, stop=False, perf_mode=DR)
                        nc.tensor.matmul(ps[:, jj, :],
                                         lhsT=kt8_s[:, 2, mc * 128:(mc + 1) * 128],
                                         rhs=qb8_s[:, 3 * g + 2, ns],
                                         start=False, stop=True)
                    es = es_p.tile([128, 2, 512], BF16)
                    nc.scalar.activation(out=es, in_=ps, func=AF.Exp,
                                         scale=1.0 / (AK * AQ))
                    with nc.allow_low_precision(reason="(E-1)*16 in fp8e4 + bf16 Z partials: validated 1.6e-3 end-to-end"):
                        nc.vector.tensor_scalar(out=et[:, 2 * j:2 * j + 2, :],
                                                in0=es, scalar1=1.0, scalar2=SE,
                                                op0=ALU.subtract, op1=ALU.mult)
                        if j == 0:
                            nc.vector.tensor_copy(out=zacc, in_=es)
                        else:
                            nc.vector.tensor_add(out=zacc, in0=zacc, in1=es)
                    for ec in range(3):
                        nc.tensor.matmul(
                            po[ec],
                            lhsT=v8_s[:, 2 * j:2 * j + 2,
                                      ec * 128:(ec + 1) * 128],
                            rhs=et[:, 2 * j:2 * j + 2, :],
                            start=(j == 0), stop=(j == 7), perf_mode=DR)
                # Z broadcast via ones-matmul; 1/(256 Z) = exp(-ln Z - ln 256)
                przb = psB.tile([128, 512], F32, tag="bb")
                for jj in range(2):
                    nc.tensor.matmul(przb, lhsT=onesb_s, rhs=zacc[:, jj, :],
                                     start=(jj == 0), stop=(jj == 1))
                lnzb = sm_p.tile([128, 512], F32, name="lnzb", tag="lnzb")
                nc.scalar.activation(out=lnzb, in_=przb, func=AF.Ln)
                rzb = sm_p.tile([128, 512], BF16)
                with nc.allow_low_precision(reason="1/Z in bf16: validated 1.5e-3 end-to-end"):
                    nc.scalar.activation(out=rzb, in_=lnzb, func=AF.Exp,
                                         scale=-1.0, bias=negln_s)
                with nc.allow_low_precision(reason="ocat bf16: validated 1.5e-3 end-to-end"):
                    for ec in range(3):
                        nc.vector.scalar_tensor_tensor(
                            out=ocat[:, 3 * g + ec, :], in0=po[ec],
                            scalar=cv_s[:, ec:ec + 1], in1=rzb,
                            op0=ALU.add, op1=ALU.mult)

            # ---- final projection + bias for this n512 chunk
            for ccp in range(3):
                pf = psA.tile([128, 2, 512], F32, tag="acc")
                for fc in range(18):
                    nc.tensor.matmul(pf[:, 0, :],
                                     lhsT=wbig_s[:, fc, ccp * 128:(ccp + 1) * 128],
                                     rhs=ocat[:, fc, :],
                                     start=(fc == 0), stop=(fc == 17))
                ot = out_p.tile([128, 512], F32)
                nc.vector.tensor_scalar_add(out=ot, in0=pf[:, 0, :],
                                            scalar1=fb_s[:, ccp:ccp + 1])
                nc.sync.dma_start(
                    out=d_out.ap()[ccp * 128:(ccp + 1) * 128, ns], in_=ot)

    nc.finalize()
    return nc


def _fold(w_qkv, b_qkv, w_l, w_w, b_w, w_proj, b_proj):
    bf = ml_dtypes.bfloat16
    Wq = w_qkv[:, :DIM].reshape(DIM, HEADS, D)
    bq = b_qkv[:DIM].reshape(HEADS, D)
    Wk = w_qkv[:, DIM:2 * DIM]
    Wv = w_qkv[:, 2 * DIM:]
    bv = b_qkv[2 * DIM:].reshape(HEADS, D)

    Wqbig = (np.einsum('chd,hg->cghd', Wq, w_l) * SCALE).reshape(DIM, HEADS * DIM)
    bqbig = (np.einsum('hd,hg->ghd', bq, w_l) * SCALE).reshape(HEADS * DIM)
    w_proj_r = w_proj.reshape(HEADS, D, DIM)
    Wbig = np.einsum('gz,zdc->gzdc', w_w, w_proj_r).reshape(HEADS * DIM, DIM)
    c_bias = (b_proj
              + np.einsum('gz,zdc,zd->c', w_w, w_proj_r, bv)
              + M * np.einsum('z,zdc,zd->c', b_w, w_proj_r, bv))
    bwexp = np.repeat(b_w, D) / AV
    return dict(wqbig=(Wqbig * AQ).astype(bf), bqbig=(bqbig * AQ).astype(np.float32),
                wk=(Wk * AK).astype(bf), wv=(Wv * AV).astype(bf),
                wbig=Wbig.astype(bf),
                wproj=w_proj.astype(bf), bwexp=bwexp.astype(np.float32),
                cbias=c_bias.astype(np.float32))


def kernel(**inputs):
    x = np.asarray(inputs["x"], np.float32)
    f = _fold(*[np.asarray(inputs[k], np.float32) for k in
                ("w_qkv", "b_qkv", "w_l", "w_w", "b_w", "w_proj", "b_proj")])

    if "nc" not in _CACHE:
        _CACHE["nc"] = build()
    nc = _CACHE["nc"]

    bf = ml_dtypes.bfloat16
    in_maps = []
    for core in range(8):
        b, half = core // 2, core % 2
        xT = np.ascontiguousarray(x[b].T).astype(bf)
        in_maps.append({
            "xt": xT,
            "xht": np.ascontiguousarray(xT[:, half * NH:(half + 1) * NH]),
            **f,
        })
    import os
    trace = bool(int(os.environ.get("BASSK_TRACE", "0")))
    res = run_bass_kernel_spmd(nc, in_maps, core_ids=list(range(8)),
                               trace=trace)
    _CACHE["last_results"] = res

    out = np.empty((B, N, DIM), np.float32)
    for core in range(8):
        b, half = core // 2, core % 2
        out[b, half * NH:(half + 1) * NH, :] = res.results[core]["out"].T
    return out


# revision 8
# speedup vs baseline: 3.7805x; 1.1755x over previous
"""Talking-heads attention (B=4, N=2048, C=384, H=6, d=64) on 8 trn2 cores.

Sharding: data-parallel over (batch b, query-half) -> 8 shards. Each core
computes attention for 1024 query rows of one batch against the full 2048
keys of that batch; tiny weights are replicated.

Algorithmic restructuring (validated exactly vs reference in numpy):
  * pre-softmax talking-heads mix w_l is folded into the Q projection:
      Qbig = x @ Wqbig + bqbig,  Wqbig[c,(g,h,d)] = w_l[h,g]*scale*Wq[c,(h,d)]
    so mixed scores are S[g] = Qbig_g @ K^T (contraction 384, full PE util).
  * key bias b_k and pre-mix bias b_l drop out (softmax row-invariance).
  * scores are tiny (|S| < ~0.1): exp with no max-subtraction.
  * post-softmax mix w_w + out-projection fold into one matrix
      Wbig[(g,(g2,d)),c'] = w_w[g,g2]*w_proj[(g2,d),c']
    applied to the per-head cross outputs O[g] = (E_g/Z_g) @ Vcat.
  * V bias + b_w colsum terms fold into a host constant + a per-batch
    device-computed correction row c_row = (b_w ⊙ colsum V) @ w_proj.

fp8 acceleration: the two dominant GEMMs (scores, A@V — 9.7 GFLOP each per
core) run in fp8e4 with perf_mode=DoubleRow (256-deep contraction per
instruction). Scales are folded into host weights: K,V are x16, Qbig x2048,
so fp8 operands sit in e4m3's sweet range. E is centered (E-1)*16 before
quantization so the attention weights' fluctuation survives fp8; the
removed DC term Σ_m V[m,:] is restored exactly in PSUM from a bf16-V
column-sum, which also cancels V's fp8 quantization error on the output's
DC component (validated: rel_l2 1.55e-3, same as the all-bf16 version).

Everything on-device runs feature-major (activations transposed), so no
PE transposes are needed anywhere: host supplies x^T, device returns out^T.
"""
import numpy as np
import ml_dtypes

import concourse.bacc as bacc
import concourse.tile as tile
import concourse.mybir as mybir
from concourse.bass_utils import run_bass_kernel_spmd

DIM = 384
HEADS = 6
D = DIM // HEADS
B, N = 4, 2048
M = N
NH = N // 2               # query rows per core
SCALE = D ** -0.5
F32 = mybir.dt.float32
BF16 = mybir.dt.bfloat16
FP8 = mybir.dt.float8e4
AF = mybir.ActivationFunctionType
ALU = mybir.AluOpType
DR = mybir.MatmulPerfMode.DoubleRow

AK = 16.0                 # fp8 scale on K   (folded into w_k on host)
AQ = 2048.0               # fp8 scale on Qbig (folded into w_qbig on host)
AV = 16.0                 # fp8 scale on V   (folded into w_v on host)
SE = 16.0                 # fp8 scale on (E - 1)

_CACHE = {}


def build():
    nc = bacc.Bacc(None, target_bir_lowering=False, debug=False)

    # ---- DRAM parameters (per-core inputs; identical program on all cores)
    d_xt = nc.dram_tensor("xt", [DIM, M], BF16, kind="ExternalInput")
    d_xht = nc.dram_tensor("xht", [DIM, NH], BF16, kind="ExternalInput")
    d_wqbig = nc.dram_tensor("wqbig", [DIM, HEADS * DIM], BF16, kind="ExternalInput")
    d_bqbig = nc.dram_tensor("bqbig", [HEADS * DIM], F32, kind="ExternalInput")
    d_wk = nc.dram_tensor("wk", [DIM, DIM], BF16, kind="ExternalInput")
    d_wv = nc.dram_tensor("wv", [DIM, DIM], BF16, kind="ExternalInput")
    d_wbig = nc.dram_tensor("wbig", [HEADS * DIM, DIM], BF16, kind="ExternalInput")
    d_wproj = nc.dram_tensor("wproj", [DIM, DIM], BF16, kind="ExternalInput")
    d_bwexp = nc.dram_tensor("bwexp", [DIM], F32, kind="ExternalInput")
    d_cbias = nc.dram_tensor("cbias", [DIM], F32, kind="ExternalInput")
    d_out = nc.dram_tensor("out", [DIM, NH], F32, kind="ExternalOutput")

    with tile.TileContext(nc) as tc, \
         tc.tile_pool(name="singles", bufs=1) as singles, \
         tc.tile_pool(name="psA", bufs=2, space="PSUM") as psA, \
         tc.tile_pool(name="psO", bufs=3, space="PSUM") as psO, \
         tc.tile_pool(name="psB", bufs=1, space="PSUM") as psB, \
         tc.tile_pool(name="et_p", bufs=2) as et_p, \
         tc.tile_pool(name="es_p", bufs=3) as es_p, \
         tc.tile_pool(name="z_p", bufs=2) as z_p, \
         tc.tile_pool(name="oc_p", bufs=2) as oc_p, \
         tc.tile_pool(name="sm_p", bufs=2) as sm_p, \
         tc.tile_pool(name="out_p", bufs=3) as out_p, \
         tc.tile_pool(name="dram", bufs=1, space="DRAM") as dram:

        # ---- load everything to SBUF (chunked feature-major layouts)
        def load(pool, dparam, shape, rearr, dt, **kw):
            t = pool.tile(shape, dt, name=dparam.name + "_s",
                          tag=dparam.name + "_s")
            nc.sync.dma_start(out=t, in_=dparam.ap().rearrange(rearr, **kw))
            return t

        xt_s = singles.tile([128, 3, M], BF16, name="xt_s", tag="xt_s")
        xht_s = singles.tile([128, 3, NH], BF16, name="xht_s", tag="xht_s")
        wqbig_s = singles.tile([128, 3, HEADS * DIM], BF16, name="wqbig_s",
                               tag="wqbig_s")
        wk_s = singles.tile([128, 3, DIM], BF16, name="wk_s", tag="wk_s")
        wv_s = singles.tile([128, 3, DIM], BF16, name="wv_s", tag="wv_s")
        wbig_s = singles.tile([128, 18, DIM], BF16, name="wbig_s", tag="wbig_s")
        wproj_s = singles.tile([128, 3, DIM], BF16, name="wproj_s",
                               tag="wproj_s")
        # DMA order = need order: wk/wv + x first (K^T, V), then Qbig's
        # operands, then the final-projection weights.
        for cc in range(3):
            nc.sync.dma_start(out=wk_s[:, cc, :],
                              in_=d_wk.ap()[cc * 128:(cc + 1) * 128, :])
            nc.sync.dma_start(out=wv_s[:, cc, :],
                              in_=d_wv.ap()[cc * 128:(cc + 1) * 128, :])
        for m5 in range(4):
            for cc in range(3):
                nc.sync.dma_start(
                    out=xt_s[:, cc, m5 * 512:(m5 + 1) * 512],
                    in_=d_xt.ap()[cc * 128:(cc + 1) * 128,
                                  m5 * 512:(m5 + 1) * 512])
        for cc in range(3):
            nc.sync.dma_start(out=xht_s[:, cc, :],
                              in_=d_xht.ap()[cc * 128:(cc + 1) * 128, :])
            nc.sync.dma_start(out=wqbig_s[:, cc, :],
                              in_=d_wqbig.ap()[cc * 128:(cc + 1) * 128, :])
        bqbig_s = load(singles, d_bqbig, [128, 18], "(fc p) -> p fc", F32, p=128)
        for fc in range(18):
            nc.sync.dma_start(out=wbig_s[:, fc, :],
                              in_=d_wbig.ap()[fc * 128:(fc + 1) * 128, :])
        for cc in range(3):
            nc.sync.dma_start(out=wproj_s[:, cc, :],
                              in_=d_wproj.ap()[cc * 128:(cc + 1) * 128, :])
        bwexp_s = load(singles, d_bwexp, [1, DIM], "(o e) -> o e", F32, o=1)
        cbias_s = load(singles, d_cbias, [128, 3], "(cc p) -> p cc", F32, p=128)

        ones_s = singles.tile([128, 1], BF16)
        nc.vector.memset(ones_s, 1.0)
        onesrow_s = singles.tile([1, 128], BF16)
        nc.vector.memset(onesrow_s, 1.0)

        kt8_s = singles.tile([128, 3, M], FP8)        # fp8 K^T  [hd, m], x16
        v_s = singles.tile([128, 16, DIM], BF16)      # bf16 V (colsum path), x16
        v8_s = singles.tile([128, 16, DIM], FP8)      # fp8 V, x16
        qb8_s = singles.tile([128, 18, NH], FP8)      # fp8 Qbig^T [(g,hd), n], x2048
        fb_s = singles.tile([128, 3], F32)            # c_row + c_bias per c'-chunk
        cv_s = singles.tile([128, 3], F32)            # 256*colsumV per c'-chunk

        # ---- prologue: K^T = Wk^T @ x^T  (x16), quantize to fp8
        for m5 in range(4):
            for fc in range(3):
                pt = psA.tile([128, 2, 512], F32, tag="acc")
                for cc in range(3):
                    nc.tensor.matmul(pt[:, 0, :],
                                     lhsT=wk_s[:, cc, fc * 128:(fc + 1) * 128],
                                     rhs=xt_s[:, cc, m5 * 512:(m5 + 1) * 512],
                                     start=(cc == 0), stop=(cc == 2))
                with nc.allow_low_precision(reason="K quantized to fp8e4 (x16 scale); validated 1.6e-3 end-to-end"):
                    nc.vector.tensor_copy(out=kt8_s[:, fc, m5 * 512:(m5 + 1) * 512],
                                          in_=pt[:, 0, :])

        # ---- prologue: V = x @ Wv (x16); bf16 copy for colsum + fp8 for AV
        for mc in range(16):
            pv = psA.tile([128, 2, 512], F32, tag="acc")
            for cc in range(3):
                nc.tensor.matmul(pv[:, 0, :DIM],
                                 lhsT=xt_s[:, cc, mc * 128:(mc + 1) * 128],
                                 rhs=wv_s[:, cc, :],
                                 start=(cc == 0), stop=(cc == 2))
            nc.scalar.copy(out=v_s[:, mc, :], in_=pv[:, 0, :DIM])
            with nc.allow_low_precision(reason="V quantized to fp8e4 (x16 scale); DC error cancelled via bf16 colsum"):
                nc.vector.tensor_copy(out=v8_s[:, mc, :], in_=pv[:, 0, :DIM])

        # ---- prologue: Qbig^T = Wqbig^T @ xh^T + bqbig (x2048), quantize fp8
        for fc in range(18):
            for n5 in range(2):
                pq = psA.tile([128, 2, 512], F32, tag="acc")
                for cc in range(3):
                    nc.tensor.matmul(pq[:, 0, :],
                                     lhsT=wqbig_s[:, cc, fc * 128:(fc + 1) * 128],
                                     rhs=xht_s[:, cc, n5 * 512:(n5 + 1) * 512],
                                     start=(cc == 0), stop=(cc == 2))
                with nc.allow_low_precision(reason="Qbig quantized to fp8e4 (x2048 scale); validated 1.6e-3 end-to-end"):
                    nc.scalar.activation(
                        out=qb8_s[:, fc, n5 * 512:(n5 + 1) * 512], in_=pq[:, 0, :],
                        func=AF.Identity, bias=bqbig_s[:, fc:fc + 1])

        # ---- S_v = colsum(V)*16;  c_row = (b_w ⊙ colsumV) @ w_proj;  fb, cv
        psv = psB.tile([1, 512], F32, tag="bb", name="psv")
        for mc in range(16):
            nc.tensor.matmul(psv[:, :DIM], lhsT=ones_s, rhs=v_s[:, mc, :],
                             start=(mc == 0), stop=(mc == 15))
        t_s = sm_p.tile([1, DIM], BF16)
        nc.vector.tensor_mul(out=t_s, in0=psv[:, :DIM], in1=bwexp_s)
        u_s = sm_p.tile([1, DIM], F32, name="u_s", tag="u_s")
        nc.vector.tensor_scalar_mul(out=u_s, in0=psv[:, :DIM], scalar1=SE)
        scr = dram.tile([1, DIM], BF16)
        nc.sync.dma_start(out=scr, in_=t_s)
        scr2 = dram.tile([1, DIM], F32, name="scr2", tag="scr2")
        nc.sync.dma_start(out=scr2, in_=u_s)
        tT_s = sm_p.tile([128, 3], BF16)
        nc.sync.dma_start(out=tT_s, in_=scr[0].rearrange("(gc p) -> p gc", p=128))
        nc.sync.dma_start(out=cv_s, in_=scr2[0].rearrange("(gc p) -> p gc", p=128))
        for ccp in range(3):
            pcr = psB.tile([128, 512], F32, tag="bb")
            for gc in range(3):
                nc.tensor.matmul(pcr[:, :1],
                                 lhsT=wproj_s[:, gc, ccp * 128:(ccp + 1) * 128],
                                 rhs=tT_s[:, gc:gc + 1],
                                 start=(gc == 0), stop=(gc == 2))
            nc.vector.tensor_scalar_add(out=fb_s[:, ccp:ccp + 1], in0=pcr[:, :1],
                                        scalar1=cbias_s[:, ccp:ccp + 1])

        # ---- attention: per (n512-chunk, mixed-head g)
        onesb_s = singles.tile([128, 128], BF16)
        nc.vector.memset(onesb_s, 1.0)
        for n5 in range(2):
            ns = slice(n5 * 512, (n5 + 1) * 512)
            ocat = oc_p.tile([128, 18, 512], BF16)
            for g in range(6):
                et = et_p.tile([128, 16, 512], FP8)   # (E-1)*16 fp8, chunk-major
                zacc = z_p.tile([128, 2, 512], BF16)  # paired partial Z sums
                po = [psO.tile([128, 512], F32, tag="po", name=f"po{_ec}")
                      for _ec in range(3)]
                for j in range(8):                    # pairs of 128-key chunks
                    ps = psA.tile([128, 2, 512], F32, tag="acc")
                    for jj in range(2):
                        mc = 2 * j + jj
                        # scores: fp8 DoubleRow c-chunks {0,1}, plain fp8 chunk 2
                        nc.tensor.matmul(ps[:, jj, :],
                                         lhsT=kt8_s[:, 0:2, mc * 128:(mc + 1) * 128],
                                         rhs=qb8_s[:, 3 * g:3 * g + 2, ns],
                                         start=True, stop=False, perf_mode=DR)
                        nc.tensor.matmul(ps[:, jj, :],
                                         lhsT=kt8_s[:, 2, mc * 128:(mc + 1) * 128],
                                         rhs=qb8_s[:, 3 * g + 2, ns],
                                         start=False, stop=True)
                    es = es_p.tile([128, 2, 512], BF16)
                    nc.scalar.activation(out=es, in_=ps, func=AF.Exp,
                                         scale=1.0 / (AK * AQ))
                    with nc.allow_low_precision(reason="(E-1)*16 in fp8e4 + bf16 Z partials: validated 1.6e-3 end-to-end"):
                        nc.vector.tensor_scalar(out=et[:, 2 * j:2 * j + 2, :],
                                                in0=es, scalar1=1.0, scalar2=SE,
                                                op0=ALU.subtract, op1=ALU.mult)
                        if j == 0:
                            nc.vector.tensor_copy(out=zacc, in_=es)
                        else:
                            nc.vector.tensor_add(out=zacc, in0=zacc, in1=es)
                    for ec in range(3):
                        nc.tensor.matmul(
                            po[ec],
                            lhsT=v8_s[:, 2 * j:2 * j + 2,
                                      ec * 128:(ec + 1) * 128],
                            rhs=et[:, 2 * j:2 * j + 2, :],
                            start=(j == 0), stop=(j == 7), perf_mode=DR)
                # Z broadcast via ones-matmul; rzb = ~1/Z (x256 folded into wbig)
                przb = psB.tile([128, 512], F32, tag="bb")
                for jj in range(2):
                    nc.tensor.matmul(przb, lhsT=onesb_s, rhs=zacc[:, jj, :],
                                     start=(jj == 0), stop=(jj == 1))
                rzb = sm_p.tile([128, 512], F32)
                nc.vector.reciprocal_approx_fast(out=rzb, in_=przb)
                with nc.allow_low_precision(reason="ocat bf16: validated 1.5e-3 end-to-end"):
                    for ec in range(3):
                        nc.vector.scalar_tensor_tensor(
                            out=ocat[:, 3 * g + ec, :], in0=po[ec],
                            scalar=cv_s[:, ec:ec + 1], in1=rzb,
                            op0=ALU.add, op1=ALU.mult)

            # ---- final projection + bias for this n512 chunk
            for ccp in range(3):
                pf = psA.tile([128, 2, 512], F32, tag="acc")
                for fc in range(18):
                    nc.tensor.matmul(pf[:, 0, :],
                                     lhsT=wbig_s[:, fc, ccp * 128:(ccp + 1) * 128],
                                     rhs=ocat[:, fc, :],
                                     start=(fc == 0), stop=(fc == 17))
                ot = out_p.tile([128, 512], F32)
                nc.vector.tensor_scalar_add(out=ot, in0=pf[:, 0, :],
                                            scalar1=fb_s[:, ccp:ccp + 1])
                nc.sync.dma_start(
                    out=d_out.ap()[ccp * 128:(ccp + 1) * 128, ns], in_=ot)

    nc.finalize()
    return nc


def _fold(w_qkv, b_qkv, w_l, w_w, b_w, w_proj, b_proj):
    bf = ml_dtypes.bfloat16
    Wq = w_qkv[:, :DIM].reshape(DIM, HEADS, D)
    bq = b_qkv[:DIM].reshape(HEADS, D)
    Wk = w_qkv[:, DIM:2 * DIM]
    Wv = w_qkv[:, 2 * DIM:]
    bv = b_qkv[2 * DIM:].reshape(HEADS, D)

    Wqbig = (np.einsum('chd,hg->cghd', Wq, w_l) * SCALE).reshape(DIM, HEADS * DIM)
    bqbig = (np.einsum('hd,hg->ghd', bq, w_l) * SCALE).reshape(HEADS * DIM)
    w_proj_r = w_proj.reshape(HEADS, D, DIM)
    Wbig = np.einsum('gz,zdc->gzdc', w_w, w_proj_r).reshape(HEADS * DIM, DIM)
    c_bias = (b_proj
              + np.einsum('gz,zdc,zd->c', w_w, w_proj_r, bv)
              + M * np.einsum('z,zdc,zd->c', b_w, w_proj_r, bv))
    bwexp = np.repeat(b_w, D) / AV
    return dict(wqbig=(Wqbig * AQ).astype(bf), bqbig=(bqbig * AQ).astype(np.float32),
                wk=(Wk * AK).astype(bf), wv=(Wv * AV).astype(bf),
                wbig=(Wbig / (AV * SE)).astype(bf),
                wproj=w_proj.astype(bf), bwexp=bwexp.astype(np.float32),
                cbias=c_bias.astype(np.float32))


def kernel(**inputs):
    x = np.asarray(inputs["x"], np.float32)
    f = _fold(*[np.asarray(inputs[k], np.float32) for k in
                ("w_qkv", "b_qkv", "w_l", "w_w", "b_w", "w_proj", "b_proj")])

    if "nc" not in _CACHE:
        _CACHE["nc"] = build()
    nc = _CACHE["nc"]

    bf = ml_dtypes.bfloat16
    in_maps = []
    for core in range(8):
        b, half = core // 2, core % 2
        xT = np.ascontiguousarray(x[b].T).astype(bf)
        in_maps.append({
            "xt": xT,
            "xht": np.ascontiguousarray(xT[:, half * NH:(half + 1) * NH]),
            **f,
        })
    import os
    trace = bool(int(os.environ.get("BASSK_TRACE", "0")))
    res = run_bass_kernel_spmd(nc, in_maps, core_ids=list(range(8)),
                               trace=trace)
    _CACHE["last_results"] = res

    out = np.empty((B, N, DIM), np.float32)
    for core in range(8):
        b, half = core // 2, core % 2
        out[b, half * NH:(half + 1) * NH, :] = res.results[core]["out"].T
    return out


# revision 10
# speedup vs baseline: 3.8072x; 1.0071x over previous
"""Talking-heads attention (B=4, N=2048, C=384, H=6, d=64) on 8 trn2 cores.

Sharding: data-parallel over (batch b, query-half) -> 8 shards. Each core
computes attention for 1024 query rows of one batch against the full 2048
keys of that batch; tiny weights are replicated.

Algorithmic restructuring (validated exactly vs reference in numpy):
  * pre-softmax talking-heads mix w_l is folded into the Q projection:
      Qbig = x @ Wqbig + bqbig,  Wqbig[c,(g,h,d)] = w_l[h,g]*scale*Wq[c,(h,d)]
    so mixed scores are S[g] = Qbig_g @ K^T (contraction 384, full PE util).
  * key bias b_k and pre-mix bias b_l drop out (softmax row-invariance).
  * scores are tiny (|S| < ~0.1): exp with no max-subtraction.
  * post-softmax mix w_w + out-projection fold into one matrix
      Wbig[(g,(g2,d)),c'] = w_w[g,g2]*w_proj[(g2,d),c']
    applied to the per-head cross outputs O[g] = (E_g/Z_g) @ Vcat.
  * V bias + b_w colsum terms fold into a host constant + a per-batch
    device-computed correction row c_row = (b_w ⊙ colsum V) @ w_proj.

fp8 acceleration: the two dominant GEMMs (scores, A@V — 9.7 GFLOP each per
core) run in fp8e4 with perf_mode=DoubleRow (256-deep contraction per
instruction). Scales are folded into host weights: K,V are x16, Qbig x2048,
so fp8 operands sit in e4m3's sweet range. E is centered (E-1)*16 before
quantization so the attention weights' fluctuation survives fp8; the
removed DC term Σ_m V[m,:] is restored exactly in PSUM from a bf16-V
column-sum, which also cancels V's fp8 quantization error on the output's
DC component (validated: rel_l2 1.55e-3, same as the all-bf16 version).

Everything on-device runs feature-major (activations transposed), so no
PE transposes are needed anywhere: host supplies x^T, device returns out^T.
"""
import numpy as np
import ml_dtypes

import concourse.bacc as bacc
import concourse.tile as tile
import concourse.mybir as mybir
from concourse.bass_utils import run_bass_kernel_spmd

DIM = 384
HEADS = 6
D = DIM // HEADS
B, N = 4, 2048
M = N
NH = N // 2               # query rows per core
SCALE = D ** -0.5
F32 = mybir.dt.float32
BF16 = mybir.dt.bfloat16
FP8 = mybir.dt.float8e4
AF = mybir.ActivationFunctionType
ALU = mybir.AluOpType
DR = mybir.MatmulPerfMode.DoubleRow

AK = 16.0                 # fp8 scale on K   (folded into w_k on host)
AQ = 2048.0               # fp8 scale on Qbig (folded into w_qbig on host)
AV = 16.0                 # fp8 scale on V   (folded into w_v on host)
SE = 16.0                 # fp8 scale on (E - 1)

_CACHE = {}


def build():
    nc = bacc.Bacc(None, target_bir_lowering=False, debug=False)

    # ---- DRAM parameters (per-core inputs; identical program on all cores)
    d_xt = nc.dram_tensor("xt", [DIM, M], BF16, kind="ExternalInput")
    d_xht = nc.dram_tensor("xht", [DIM, NH], BF16, kind="ExternalInput")
    d_wqbig = nc.dram_tensor("wqbig", [DIM, HEADS * DIM], BF16, kind="ExternalInput")
    d_bqbig = nc.dram_tensor("bqbig", [HEADS * DIM], F32, kind="ExternalInput")
    d_wk = nc.dram_tensor("wk", [DIM, DIM], BF16, kind="ExternalInput")
    d_wv = nc.dram_tensor("wv", [DIM, DIM], BF16, kind="ExternalInput")
    d_wbig = nc.dram_tensor("wbig", [HEADS * DIM, DIM], BF16, kind="ExternalInput")
    d_wproj = nc.dram_tensor("wproj", [DIM, DIM], BF16, kind="ExternalInput")
    d_bwexp = nc.dram_tensor("bwexp", [DIM], F32, kind="ExternalInput")
    d_cbias = nc.dram_tensor("cbias", [DIM], F32, kind="ExternalInput")
    d_out = nc.dram_tensor("out", [DIM, NH], F32, kind="ExternalOutput")

    with tile.TileContext(nc) as tc, \
         tc.tile_pool(name="singles", bufs=1) as singles, \
         tc.tile_pool(name="psA", bufs=2, space="PSUM") as psA, \
         tc.tile_pool(name="psO", bufs=3, space="PSUM") as psO, \
         tc.tile_pool(name="psB", bufs=1, space="PSUM") as psB, \
         tc.tile_pool(name="et_p", bufs=2) as et_p, \
         tc.tile_pool(name="es_p", bufs=3) as es_p, \
         tc.tile_pool(name="z_p", bufs=2) as z_p, \
         tc.tile_pool(name="oc_p", bufs=2) as oc_p, \
         tc.tile_pool(name="sm_p", bufs=2) as sm_p, \
         tc.tile_pool(name="out_p", bufs=3) as out_p, \
         tc.tile_pool(name="dram", bufs=1, space="DRAM") as dram:

        # ---- load everything to SBUF (chunked feature-major layouts)
        def load(pool, dparam, shape, rearr, dt, **kw):
            t = pool.tile(shape, dt, name=dparam.name + "_s",
                          tag=dparam.name + "_s")
            nc.sync.dma_start(out=t, in_=dparam.ap().rearrange(rearr, **kw))
            return t

        xt_s = singles.tile([128, 3, M], BF16, name="xt_s", tag="xt_s")
        xht_s = singles.tile([128, 3, NH], BF16, name="xht_s", tag="xht_s")
        wqbig_s = singles.tile([128, 3, HEADS * DIM], BF16, name="wqbig_s",
                               tag="wqbig_s")
        wk_s = singles.tile([128, 3, DIM], BF16, name="wk_s", tag="wk_s")
        wv_s = singles.tile([128, 3, DIM], BF16, name="wv_s", tag="wv_s")
        wbig_s = singles.tile([128, 18, DIM], BF16, name="wbig_s", tag="wbig_s")
        wproj_s = singles.tile([128, 3, DIM], BF16, name="wproj_s",
                               tag="wproj_s")
        # DMA order = need order: wk/wv + x first (K^T, V), then Qbig's
        # operands, then the final-projection weights.
        for cc in range(3):
            nc.sync.dma_start(out=wk_s[:, cc, :],
                              in_=d_wk.ap()[cc * 128:(cc + 1) * 128, :])
            nc.sync.dma_start(out=wv_s[:, cc, :],
                              in_=d_wv.ap()[cc * 128:(cc + 1) * 128, :])
        for m5 in range(4):
            for cc in range(3):
                nc.sync.dma_start(
                    out=xt_s[:, cc, m5 * 512:(m5 + 1) * 512],
                    in_=d_xt.ap()[cc * 128:(cc + 1) * 128,
                                  m5 * 512:(m5 + 1) * 512])
        for cc in range(3):
            nc.sync.dma_start(out=xht_s[:, cc, :],
                              in_=d_xht.ap()[cc * 128:(cc + 1) * 128, :])
            nc.sync.dma_start(out=wqbig_s[:, cc, :],
                              in_=d_wqbig.ap()[cc * 128:(cc + 1) * 128, :])
        bqbig_s = load(singles, d_bqbig, [128, 18], "(fc p) -> p fc", F32, p=128)
        for fc in range(18):
            nc.sync.dma_start(out=wbig_s[:, fc, :],
                              in_=d_wbig.ap()[fc * 128:(fc + 1) * 128, :])
        for cc in range(3):
            nc.sync.dma_start(out=wproj_s[:, cc, :],
                              in_=d_wproj.ap()[cc * 128:(cc + 1) * 128, :])
        bwexp_s = load(singles, d_bwexp, [1, DIM], "(o e) -> o e", F32, o=1)
        cbias_s = load(singles, d_cbias, [128, 3], "(cc p) -> p cc", F32, p=128)

        ones_s = singles.tile([128, 1], BF16)
        nc.vector.memset(ones_s, 1.0)
        onesrow_s = singles.tile([1, 128], BF16)
        nc.vector.memset(onesrow_s, 1.0)

        kt8_s = singles.tile([128, 3, M], FP8)        # fp8 K^T  [hd, m], x16
        v_s = singles.tile([128, 16, DIM], BF16)      # bf16 V (colsum path), x16
        v8_s = singles.tile([128, 16, DIM], FP8)      # fp8 V, x16
        qb8_s = singles.tile([128, 18, NH], FP8)      # fp8 Qbig^T [(g,hd), n], x2048
        fb_s = singles.tile([128, 3], F32)            # c_row + c_bias per c'-chunk
        cv_s = singles.tile([128, 3], F32)            # 256*colsumV per c'-chunk

        # ---- prologue: K^T = Wk^T @ x^T  (x16), quantize to fp8
        for m10 in range(2):
            for fc in range(3):
                pt = psA.tile([128, 2, 512], F32, tag="acc")
                for jj in range(2):
                    for cc in range(3):
                        nc.tensor.matmul(
                            pt[:, jj, :],
                            lhsT=wk_s[:, cc, fc * 128:(fc + 1) * 128],
                            rhs=xt_s[:, cc, (2 * m10 + jj) * 512:
                                     (2 * m10 + jj + 1) * 512],
                            start=(cc == 0), stop=(cc == 2))
                with nc.allow_low_precision(reason="K quantized to fp8e4 (x16 scale); validated 1.6e-3 end-to-end"):
                    nc.vector.tensor_copy(
                        out=kt8_s[:, fc, m10 * 1024:(m10 + 1) * 1024], in_=pt)

        # ---- prologue: V = x @ Wv (x16); bf16 copy for colsum + fp8 for AV
        for mc in range(16):
            pv = psO.tile([128, 512], F32, tag="po", name="pv")
            for cc in range(3):
                nc.tensor.matmul(pv[:, :DIM],
                                 lhsT=xt_s[:, cc, mc * 128:(mc + 1) * 128],
                                 rhs=wv_s[:, cc, :],
                                 start=(cc == 0), stop=(cc == 2))
            nc.scalar.copy(out=v_s[:, mc, :], in_=pv[:, :DIM])
            with nc.allow_low_precision(reason="V quantized to fp8e4 (x16 scale); DC error cancelled via bf16 colsum"):
                nc.vector.tensor_copy(out=v8_s[:, mc, :], in_=pv[:, :DIM])


        # ---- S_v = colsum(V)*16;  c_row = (b_w ⊙ colsumV) @ w_proj;  fb, cv
        psv = psB.tile([1, 512], F32, tag="bb", name="psv")
        for mc in range(16):
            nc.tensor.matmul(psv[:, :DIM], lhsT=ones_s, rhs=v_s[:, mc, :],
                             start=(mc == 0), stop=(mc == 15))
        t_s = sm_p.tile([1, DIM], BF16)
        nc.vector.tensor_mul(out=t_s, in0=psv[:, :DIM], in1=bwexp_s)
        u_s = sm_p.tile([1, DIM], F32, name="u_s", tag="u_s")
        nc.vector.tensor_scalar_mul(out=u_s, in0=psv[:, :DIM], scalar1=SE)
        scr = dram.tile([1, DIM], BF16)
        nc.sync.dma_start(out=scr, in_=t_s)
        scr2 = dram.tile([1, DIM], F32, name="scr2", tag="scr2")
        nc.sync.dma_start(out=scr2, in_=u_s)
        tT_s = sm_p.tile([128, 3], BF16)
        nc.sync.dma_start(out=tT_s, in_=scr[0].rearrange("(gc p) -> p gc", p=128))
        nc.sync.dma_start(out=cv_s, in_=scr2[0].rearrange("(gc p) -> p gc", p=128))
        for ccp in range(3):
            pcr = psB.tile([128, 512], F32, tag="bb")
            for gc in range(3):
                nc.tensor.matmul(pcr[:, :1],
                                 lhsT=wproj_s[:, gc, ccp * 128:(ccp + 1) * 128],
                                 rhs=tT_s[:, gc:gc + 1],
                                 start=(gc == 0), stop=(gc == 2))
            nc.vector.tensor_scalar_add(out=fb_s[:, ccp:ccp + 1], in0=pcr[:, :1],
                                        scalar1=cbias_s[:, ccp:ccp + 1])

        # ---- attention: per (n512-chunk, mixed-head g)
        onesb_s = singles.tile([128, 128], BF16)
        nc.vector.memset(onesb_s, 1.0)
        for n5 in range(2):
            ns = slice(n5 * 512, (n5 + 1) * 512)
            ocat = oc_p.tile([128, 18, 512], BF16)
            for g in range(6):
                if n5 == 0:
                    # Qbig for this g (both query halves), overlapped with
                    # the PE-heavy attention phase: Qbig^T = Wqbig^T@xh^T+b
                    for fl in range(3):
                        fc = 3 * g + fl
                        pq = psA.tile([128, 2, 512], F32, tag="acc", name="pq")
                        for jj in range(2):
                            for cc in range(3):
                                nc.tensor.matmul(
                                    pq[:, jj, :],
                                    lhsT=wqbig_s[:, cc, fc * 128:(fc + 1) * 128],
                                    rhs=xht_s[:, cc, jj * 512:(jj + 1) * 512],
                                    start=(cc == 0), stop=(cc == 2))
                        with nc.allow_low_precision(reason="Qbig quantized to fp8e4 (x2048 scale); validated 1.6e-3 end-to-end"):
                            nc.scalar.activation(
                                out=qb8_s[:, fc, :], in_=pq,
                                func=AF.Identity, bias=bqbig_s[:, fc:fc + 1])
                et = et_p.tile([128, 16, 512], FP8)   # (E-1)*16 fp8, chunk-major
                zacc = z_p.tile([128, 2, 512], BF16)  # paired partial Z sums
                po = [psO.tile([128, 512], F32, tag="po", name=f"po{_ec}")
                      for _ec in range(3)]
                for j in range(8):                    # pairs of 128-key chunks
                    ps = psA.tile([128, 2, 512], F32, tag="acc")
                    for jj in range(2):
                        mc = 2 * j + jj
                        # scores: fp8 DoubleRow c-chunks {0,1}, plain fp8 chunk 2
                        nc.tensor.matmul(ps[:, jj, :],
                                         lhsT=kt8_s[:, 0:2, mc * 128:(mc + 1) * 128],
                                         rhs=qb8_s[:, 3 * g:3 * g + 2, ns],
                                         start=True, stop=False, perf_mode=DR)
                        nc.tensor.matmul(ps[:, jj, :],
                                         lhsT=kt8_s[:, 2, mc * 128:(mc + 1) * 128],
                                         rhs=qb8_s[:, 3 * g + 2, ns],
                                         start=False, stop=True)
                    es = es_p.tile([128, 2, 512], BF16)
                    nc.scalar.activation(out=es, in_=ps, func=AF.Exp,
                                         scale=1.0 / (AK * AQ))
                    with nc.allow_low_precision(reason="(E-1)*16 in fp8e4 + bf16 Z partials: validated 1.6e-3 end-to-end"):
                        nc.vector.tensor_scalar(out=et[:, 2 * j:2 * j + 2, :],
                                                in0=es, scalar1=1.0, scalar2=SE,
                                                op0=ALU.subtract, op1=ALU.mult)
                        if j == 0:
                            nc.vector.tensor_copy(out=zacc, in_=es)
                        else:
                            nc.vector.tensor_add(out=zacc, in0=zacc, in1=es)
                    for ec in range(3):
                        nc.tensor.matmul(
                            po[ec],
                            lhsT=v8_s[:, 2 * j:2 * j + 2,
                                      ec * 128:(ec + 1) * 128],
                            rhs=et[:, 2 * j:2 * j + 2, :],
                            start=(j == 0), stop=(j == 7), perf_mode=DR)
                # Z broadcast via ones-matmul; rzb = ~1/Z (x256 folded into wbig)
                przb = psB.tile([128, 512], F32, tag="bb")
                for jj in range(2):
                    nc.tensor.matmul(przb, lhsT=onesb_s, rhs=zacc[:, jj, :],
                                     start=(jj == 0), stop=(jj == 1))
                rzb = sm_p.tile([128, 512], F32)
                nc.vector.reciprocal_approx_fast(out=rzb, in_=przb)
                with nc.allow_low_precision(reason="ocat bf16: validated 1.5e-3 end-to-end"):
                    for ec in range(3):
                        nc.vector.scalar_tensor_tensor(
                            out=ocat[:, 3 * g + ec, :], in0=po[ec],
                            scalar=cv_s[:, ec:ec + 1], in1=rzb,
                            op0=ALU.add, op1=ALU.mult)

            # ---- final projection + bias for this n512 chunk
            for ccp in range(3):
                pf = psA.tile([128, 2, 512], F32, tag="acc")
                for fc in range(18):
                    nc.tensor.matmul(pf[:, 0, :],
                                     lhsT=wbig_s[:, fc, ccp * 128:(ccp + 1) * 128],
                                     rhs=ocat[:, fc, :],
                                     start=(fc == 0), stop=(fc == 17))
                ot = out_p.tile([128, 512], F32)
                nc.vector.tensor_scalar_add(out=ot, in0=pf[:, 0, :],
                                            scalar1=fb_s[:, ccp:ccp + 1])
                nc.sync.dma_start(
                    out=d_out.ap()[ccp * 128:(ccp + 1) * 128, ns], in_=ot)

    nc.finalize()
    return nc


def _fold(w_qkv, b_qkv, w_l, w_w, b_w, w_proj, b_proj):
    bf = ml_dtypes.bfloat16
    Wq = w_qkv[:, :DIM].reshape(DIM, HEADS, D)
    bq = b_qkv[:DIM].reshape(HEADS, D)
    Wk = w_qkv[:, DIM:2 * DIM]
    Wv = w_qkv[:, 2 * DIM:]
    bv = b_qkv[2 * DIM:].reshape(HEADS, D)

    Wqbig = (np.einsum('chd,hg->cghd', Wq, w_l) * SCALE).reshape(DIM, HEADS * DIM)
    bqbig = (np.einsum('hd,hg->ghd', bq, w_l) * SCALE).reshape(HEADS * DIM)
    w_proj_r = w_proj.reshape(HEADS, D, DIM)
    Wbig = np.einsum('gz,zdc->gzdc', w_w, w_proj_r).reshape(HEADS * DIM, DIM)
    c_bias = (b_proj
              + np.einsum('gz,zdc,zd->c', w_w, w_proj_r, bv)
              + M * np.einsum('z,zdc,zd->c', b_w, w_proj_r, bv))
    bwexp = np.repeat(b_w, D) / AV
    return dict(wqbig=(Wqbig * AQ).astype(bf), bqbig=(bqbig * AQ).astype(np.float32),
                wk=(Wk * AK).astype(bf), wv=(Wv * AV).astype(bf),
                wbig=(Wbig / (AV * SE)).astype(bf),
                wproj=w_proj.astype(bf), bwexp=bwexp.astype(np.float32),
                cbias=c_bias.astype(np.float32))


def kernel(**inputs):
    x = np.asarray(inputs["x"], np.float32)
    f = _fold(*[np.asarray(inputs[k], np.float32) for k in
                ("w_qkv", "b_qkv", "w_l", "w_w", "b_w", "w_proj", "b_proj")])

    if "nc" not in _CACHE:
        _CACHE["nc"] = build()
    nc = _CACHE["nc"]

    bf = ml_dtypes.bfloat16
    in_maps = []
    for core in range(8):
        b, half = core // 2, core % 2
        xT = np.ascontiguousarray(x[b].T).astype(bf)
        in_maps.append({
            "xt": xT,
            "xht": np.ascontiguousarray(xT[:, half * NH:(half + 1) * NH]),
            **f,
        })
    import os
    trace = bool(int(os.environ.get("BASSK_TRACE", "0")))
    res = run_bass_kernel_spmd(nc, in_maps, core_ids=list(range(8)),
                               trace=trace)
    _CACHE["last_results"] = res

    out = np.empty((B, N, DIM), np.float32)
    for core in range(8):
        b, half = core // 2, core % 2
        out[b, half * NH:(half + 1) * NH, :] = res.results[core]["out"].T
    return out


# revision 11
# speedup vs baseline: 3.9694x; 1.0426x over previous
"""Talking-heads attention (B=4, N=2048, C=384, H=6, d=64) on 8 trn2 cores.

Sharding: data-parallel over (batch b, query-half) -> 8 shards. Each core
computes attention for 1024 query rows of one batch against the full 2048
keys of that batch; tiny weights are replicated.

Algorithmic restructuring (validated exactly vs reference in numpy):
  * pre-softmax talking-heads mix w_l is folded into the Q projection:
      Qbig = x @ Wqbig + bqbig,  Wqbig[c,(g,h,d)] = w_l[h,g]*scale*Wq[c,(h,d)]
    so mixed scores are S[g] = Qbig_g @ K^T (contraction 384, full PE util).
  * key bias b_k and pre-mix bias b_l drop out (softmax row-invariance).
  * scores are tiny (|S| < ~0.1): exp with no max-subtraction.
  * post-softmax mix w_w + out-projection fold into one matrix
      Wbig[(g,(g2,d)),c'] = w_w[g,g2]*w_proj[(g2,d),c']
    applied to the per-head cross outputs O[g] = (E_g/Z_g) @ Vcat.
  * V bias + b_w colsum terms fold into a host constant + a per-batch
    device-computed correction row c_row = (b_w ⊙ colsum V) @ w_proj.

fp8 acceleration: the two dominant GEMMs (scores, A@V — 9.7 GFLOP each per
core) run in fp8e4 with perf_mode=DoubleRow (256-deep contraction per
instruction). Scales are folded into host weights: K,V are x16, Qbig x2048,
so fp8 operands sit in e4m3's sweet range. E is centered (E-1)*16 before
quantization so the attention weights' fluctuation survives fp8; the
removed DC term Σ_m V[m,:] is restored exactly in PSUM from a bf16-V
column-sum, which also cancels V's fp8 quantization error on the output's
DC component (validated: rel_l2 1.55e-3, same as the all-bf16 version).

Everything on-device runs feature-major (activations transposed), so no
PE transposes are needed anywhere: host supplies x^T, device returns out^T.
"""
import numpy as np
import ml_dtypes

import concourse.bacc as bacc
import concourse.tile as tile
import concourse.mybir as mybir
from concourse.bass_utils import run_bass_kernel_spmd

DIM = 384
HEADS = 6
D = DIM // HEADS
B, N = 4, 2048
M = N
NH = N // 2               # query rows per core
SCALE = D ** -0.5
F32 = mybir.dt.float32
BF16 = mybir.dt.bfloat16
FP8 = mybir.dt.float8e4
AF = mybir.ActivationFunctionType
ALU = mybir.AluOpType
DR = mybir.MatmulPerfMode.DoubleRow

AK = 16.0                 # fp8 scale on K   (folded into w_k on host)
AQ = 2048.0               # fp8 scale on Qbig (folded into w_qbig on host)
AV = 16.0                 # fp8 scale on V   (folded into w_v on host)
SE = 16.0                 # fp8 scale on (E - 1)

_CACHE = {}


def build():
    nc = bacc.Bacc(None, target_bir_lowering=False, debug=False)

    # ---- DRAM parameters (per-core inputs; identical program on all cores)
    d_xt = nc.dram_tensor("xt", [DIM, M], BF16, kind="ExternalInput")
    d_xh8 = nc.dram_tensor("xh8", [DIM, NH], FP8, kind="ExternalInput")
    d_wqb8 = nc.dram_tensor("wqb8", [DIM, HEADS * DIM], FP8, kind="ExternalInput")
    d_bqbig = nc.dram_tensor("bqbig", [HEADS * DIM], F32, kind="ExternalInput")
    d_wk = nc.dram_tensor("wk", [DIM, DIM], BF16, kind="ExternalInput")
    d_wv = nc.dram_tensor("wv", [DIM, DIM], BF16, kind="ExternalInput")
    d_wbig = nc.dram_tensor("wbig", [HEADS * DIM, DIM], BF16, kind="ExternalInput")
    d_wproj = nc.dram_tensor("wproj", [DIM, DIM], BF16, kind="ExternalInput")
    d_bwexp = nc.dram_tensor("bwexp", [DIM], F32, kind="ExternalInput")
    d_cbias = nc.dram_tensor("cbias", [DIM], F32, kind="ExternalInput")
    d_out = nc.dram_tensor("out", [DIM, NH], F32, kind="ExternalOutput")

    with tile.TileContext(nc) as tc, \
         tc.tile_pool(name="singles", bufs=1) as singles, \
         tc.tile_pool(name="psA", bufs=2, space="PSUM") as psA, \
         tc.tile_pool(name="psO", bufs=3, space="PSUM") as psO, \
         tc.tile_pool(name="psB", bufs=1, space="PSUM") as psB, \
         tc.tile_pool(name="et_p", bufs=2) as et_p, \
         tc.tile_pool(name="es_p", bufs=3) as es_p, \
         tc.tile_pool(name="z_p", bufs=2) as z_p, \
         tc.tile_pool(name="oc_p", bufs=2) as oc_p, \
         tc.tile_pool(name="sm_p", bufs=2) as sm_p, \
         tc.tile_pool(name="out_p", bufs=3) as out_p, \
         tc.tile_pool(name="dram", bufs=1, space="DRAM") as dram:

        # ---- load everything to SBUF (chunked feature-major layouts)
        def load(pool, dparam, shape, rearr, dt, **kw):
            t = pool.tile(shape, dt, name=dparam.name + "_s",
                          tag=dparam.name + "_s")
            nc.sync.dma_start(out=t, in_=dparam.ap().rearrange(rearr, **kw))
            return t

        xt_s = singles.tile([128, 3, M], BF16, name="xt_s", tag="xt_s")
        xh8_s = singles.tile([128, 3, NH], FP8, name="xh8_s", tag="xh8_s")
        wqb8_s = singles.tile([128, 3, HEADS * DIM], FP8, name="wqb8_s",
                              tag="wqb8_s")
        wk_s = singles.tile([128, 3, DIM], BF16, name="wk_s", tag="wk_s")
        wv_s = singles.tile([128, 3, DIM], BF16, name="wv_s", tag="wv_s")
        wbig_s = singles.tile([128, 18, DIM], BF16, name="wbig_s", tag="wbig_s")
        wproj_s = singles.tile([128, 3, DIM], BF16, name="wproj_s",
                               tag="wproj_s")
        # DMA order = need order: wk/wv + x first (K^T, V), then Qbig's
        # operands, then the final-projection weights.
        for cc in range(3):
            nc.sync.dma_start(out=wk_s[:, cc, :],
                              in_=d_wk.ap()[cc * 128:(cc + 1) * 128, :])
            nc.sync.dma_start(out=wv_s[:, cc, :],
                              in_=d_wv.ap()[cc * 128:(cc + 1) * 128, :])
        for m5 in range(4):
            for cc in range(3):
                nc.sync.dma_start(
                    out=xt_s[:, cc, m5 * 512:(m5 + 1) * 512],
                    in_=d_xt.ap()[cc * 128:(cc + 1) * 128,
                                  m5 * 512:(m5 + 1) * 512])
        for cc in range(3):
            nc.sync.dma_start(out=xh8_s[:, cc, :],
                              in_=d_xh8.ap()[cc * 128:(cc + 1) * 128, :])
            nc.sync.dma_start(out=wqb8_s[:, cc, :],
                              in_=d_wqb8.ap()[cc * 128:(cc + 1) * 128, :])
        bqbig_s = load(singles, d_bqbig, [128, 18], "(fc p) -> p fc", F32, p=128)
        for fc in range(18):
            nc.sync.dma_start(out=wbig_s[:, fc, :],
                              in_=d_wbig.ap()[fc * 128:(fc + 1) * 128, :])
        for cc in range(3):
            nc.sync.dma_start(out=wproj_s[:, cc, :],
                              in_=d_wproj.ap()[cc * 128:(cc + 1) * 128, :])
        bwexp_s = load(singles, d_bwexp, [1, DIM], "(o e) -> o e", F32, o=1)
        cbias_s = load(singles, d_cbias, [128, 3], "(cc p) -> p cc", F32, p=128)

        ones_s = singles.tile([128, 1], BF16)
        nc.vector.memset(ones_s, 1.0)
        onesrow_s = singles.tile([1, 128], BF16)
        nc.vector.memset(onesrow_s, 1.0)

        kt8_s = singles.tile([128, 3, M], FP8)        # fp8 K^T  [hd, m], x16
        v_s = singles.tile([128, 16, DIM], BF16)      # bf16 V (colsum path), x16
        v8_s = singles.tile([128, 16, DIM], FP8)      # fp8 V, x16
        qb8_s = singles.tile([128, 18, NH], FP8)      # fp8 Qbig^T [(g,hd), n], x2048
        fb_s = singles.tile([128, 3], F32)            # c_row + c_bias per c'-chunk
        cv_s = singles.tile([128, 3], F32)            # 256*colsumV per c'-chunk

        # ---- prologue: K^T = Wk^T @ x^T  (x16), quantize to fp8
        for m10 in range(2):
            for fc in range(3):
                pt = psA.tile([128, 2, 512], F32, tag="acc")
                for jj in range(2):
                    for cc in range(3):
                        nc.tensor.matmul(
                            pt[:, jj, :],
                            lhsT=wk_s[:, cc, fc * 128:(fc + 1) * 128],
                            rhs=xt_s[:, cc, (2 * m10 + jj) * 512:
                                     (2 * m10 + jj + 1) * 512],
                            start=(cc == 0), stop=(cc == 2))
                with nc.allow_low_precision(reason="K quantized to fp8e4 (x16 scale); validated 1.6e-3 end-to-end"):
                    nc.vector.tensor_copy(
                        out=kt8_s[:, fc, m10 * 1024:(m10 + 1) * 1024], in_=pt)

        # ---- prologue: V = x @ Wv (x16); bf16 copy for colsum + fp8 for AV
        for mc in range(16):
            pv = psO.tile([128, 512], F32, tag="po", name="pv")
            for cc in range(3):
                nc.tensor.matmul(pv[:, :DIM],
                                 lhsT=xt_s[:, cc, mc * 128:(mc + 1) * 128],
                                 rhs=wv_s[:, cc, :],
                                 start=(cc == 0), stop=(cc == 2))
            nc.scalar.copy(out=v_s[:, mc, :], in_=pv[:, :DIM])
            with nc.allow_low_precision(reason="V quantized to fp8e4 (x16 scale); DC error cancelled via bf16 colsum"):
                nc.vector.tensor_copy(out=v8_s[:, mc, :], in_=pv[:, :DIM])


        # ---- S_v = colsum(V)*16;  c_row = (b_w ⊙ colsumV) @ w_proj;  fb, cv
        psv = psB.tile([1, 512], F32, tag="bb", name="psv")
        for mc in range(16):
            nc.tensor.matmul(psv[:, :DIM], lhsT=ones_s, rhs=v_s[:, mc, :],
                             start=(mc == 0), stop=(mc == 15))
        t_s = sm_p.tile([1, DIM], BF16)
        nc.vector.tensor_mul(out=t_s, in0=psv[:, :DIM], in1=bwexp_s)
        u_s = sm_p.tile([1, DIM], F32, name="u_s", tag="u_s")
        nc.vector.tensor_scalar_mul(out=u_s, in0=psv[:, :DIM], scalar1=SE)
        scr = dram.tile([1, DIM], BF16)
        nc.sync.dma_start(out=scr, in_=t_s)
        scr2 = dram.tile([1, DIM], F32, name="scr2", tag="scr2")
        nc.sync.dma_start(out=scr2, in_=u_s)
        tT_s = sm_p.tile([128, 3], BF16)
        nc.sync.dma_start(out=tT_s, in_=scr[0].rearrange("(gc p) -> p gc", p=128))
        nc.sync.dma_start(out=cv_s, in_=scr2[0].rearrange("(gc p) -> p gc", p=128))
        for ccp in range(3):
            pcr = psB.tile([128, 512], F32, tag="bb")
            for gc in range(3):
                nc.tensor.matmul(pcr[:, :1],
                                 lhsT=wproj_s[:, gc, ccp * 128:(ccp + 1) * 128],
                                 rhs=tT_s[:, gc:gc + 1],
                                 start=(gc == 0), stop=(gc == 2))
            nc.vector.tensor_scalar_add(out=fb_s[:, ccp:ccp + 1], in0=pcr[:, :1],
                                        scalar1=cbias_s[:, ccp:ccp + 1])

        # ---- attention: per (n512-chunk, mixed-head g)
        onesb_s = singles.tile([128, 128], BF16)
        nc.vector.memset(onesb_s, 1.0)
        for n5 in range(2):
            ns = slice(n5 * 512, (n5 + 1) * 512)
            ocat = oc_p.tile([128, 18, 512], BF16)
            def emit_qbig(gq):
                # Qbig for head gq (both query halves), emitted one head
                # ahead so its ACT conversion never gates the score matmuls.
                # fp8 DR inputs are x16384*x4-scaled; x1/32 rescales to x2048.
                for fl in range(3):
                    fc = 3 * gq + fl
                    pq = psA.tile([128, 2, 512], F32, tag="acc", name="pq")
                    for jj in range(2):
                        nc.tensor.matmul(
                            pq[:, jj, :],
                            lhsT=wqb8_s[:, 0:2, fc * 128:(fc + 1) * 128],
                            rhs=xh8_s[:, 0:2, jj * 512:(jj + 1) * 512],
                            start=True, stop=False, perf_mode=DR)
                        nc.tensor.matmul(
                            pq[:, jj, :],
                            lhsT=wqb8_s[:, 2, fc * 128:(fc + 1) * 128],
                            rhs=xh8_s[:, 2, jj * 512:(jj + 1) * 512],
                            start=False, stop=True)
                    with nc.allow_low_precision(reason="Qbig quantized to fp8e4 (x2048 scale); validated 1.6e-3 end-to-end"):
                        nc.scalar.activation(
                            out=qb8_s[:, fc, :], in_=pq,
                            func=AF.Identity, scale=1.0 / 32.0,
                            bias=bqbig_s[:, fc:fc + 1])

            for g in range(6):
                if n5 == 0:
                    if g == 0:
                        emit_qbig(0)
                    if g < 5:
                        emit_qbig(g + 1)
                et = et_p.tile([128, 16, 512], FP8)   # (E-1)*16 fp8, chunk-major
                zacc = z_p.tile([128, 2, 512], BF16)  # paired partial Z sums
                po = [psO.tile([128, 512], F32, tag="po", name=f"po{_ec}")
                      for _ec in range(3)]
                for j in range(8):                    # pairs of 128-key chunks
                    ps = psA.tile([128, 2, 512], F32, tag="acc")
                    for jj in range(2):
                        mc = 2 * j + jj
                        # scores: fp8 DoubleRow c-chunks {0,1}, plain fp8 chunk 2
                        nc.tensor.matmul(ps[:, jj, :],
                                         lhsT=kt8_s[:, 0:2, mc * 128:(mc + 1) * 128],
                                         rhs=qb8_s[:, 3 * g:3 * g + 2, ns],
                                         start=True, stop=False, perf_mode=DR)
                        nc.tensor.matmul(ps[:, jj, :],
                                         lhsT=kt8_s[:, 2, mc * 128:(mc + 1) * 128],
                                         rhs=qb8_s[:, 3 * g + 2, ns],
                                         start=False, stop=True)
                    es = es_p.tile([128, 2, 512], BF16)
                    nc.scalar.activation(out=es, in_=ps, func=AF.Exp,
                                         scale=1.0 / (AK * AQ))
                    with nc.allow_low_precision(reason="(E-1)*16 in fp8e4 + bf16 Z partials: validated 1.6e-3 end-to-end"):
                        nc.vector.tensor_scalar(out=et[:, 2 * j:2 * j + 2, :],
                                                in0=es, scalar1=1.0, scalar2=SE,
                                                op0=ALU.subtract, op1=ALU.mult)
                        if j == 0:
                            nc.vector.tensor_copy(out=zacc, in_=es)
                        else:
                            nc.vector.tensor_add(out=zacc, in0=zacc, in1=es)
                    for ec in range(3):
                        nc.tensor.matmul(
                            po[ec],
                            lhsT=v8_s[:, 2 * j:2 * j + 2,
                                      ec * 128:(ec + 1) * 128],
                            rhs=et[:, 2 * j:2 * j + 2, :],
                            start=(j == 0), stop=(j == 7), perf_mode=DR)
                # Z broadcast via ones-matmul; rzb = ~1/Z (x256 folded into wbig)
                przb = psB.tile([128, 512], F32, tag="bb")
                for jj in range(2):
                    nc.tensor.matmul(przb, lhsT=onesb_s, rhs=zacc[:, jj, :],
                                     start=(jj == 0), stop=(jj == 1))
                rzb = sm_p.tile([128, 512], F32)
                nc.vector.reciprocal_approx_fast(out=rzb, in_=przb)
                with nc.allow_low_precision(reason="ocat bf16: validated 1.5e-3 end-to-end"):
                    for ec in range(3):
                        nc.vector.scalar_tensor_tensor(
                            out=ocat[:, 3 * g + ec, :], in0=po[ec],
                            scalar=cv_s[:, ec:ec + 1], in1=rzb,
                            op0=ALU.add, op1=ALU.mult)

            # ---- final projection + bias for this n512 chunk
            for ccp in range(3):
                pf = psA.tile([128, 2, 512], F32, tag="acc")
                for fc in range(18):
                    nc.tensor.matmul(pf[:, 0, :],
                                     lhsT=wbig_s[:, fc, ccp * 128:(ccp + 1) * 128],
                                     rhs=ocat[:, fc, :],
                                     start=(fc == 0), stop=(fc == 17))
                ot = out_p.tile([128, 512], F32)
                nc.vector.tensor_scalar_add(out=ot, in0=pf[:, 0, :],
                                            scalar1=fb_s[:, ccp:ccp + 1])
                nc.sync.dma_start(
                    out=d_out.ap()[ccp * 128:(ccp + 1) * 128, ns], in_=ot)

    nc.finalize()
    return nc


def _fold(w_qkv, b_qkv, w_l, w_w, b_w, w_proj, b_proj):
    bf = ml_dtypes.bfloat16
    Wq = w_qkv[:, :DIM].reshape(DIM, HEADS, D)
    bq = b_qkv[:DIM].reshape(HEADS, D)
    Wk = w_qkv[:, DIM:2 * DIM]
    Wv = w_qkv[:, 2 * DIM:]
    bv = b_qkv[2 * DIM:].reshape(HEADS, D)

    Wqbig = (np.einsum('chd,hg->cghd', Wq, w_l) * SCALE).reshape(DIM, HEADS * DIM)
    bqbig = (np.einsum('hd,hg->ghd', bq, w_l) * SCALE).reshape(HEADS * DIM)
    w_proj_r = w_proj.reshape(HEADS, D, DIM)
    Wbig = np.einsum('gz,zdc->gzdc', w_w, w_proj_r).reshape(HEADS * DIM, DIM)
    c_bias = (b_proj
              + np.einsum('gz,zdc,zd->c', w_w, w_proj_r, bv)
              + M * np.einsum('z,zdc,zd->c', b_w, w_proj_r, bv))
    bwexp = np.repeat(b_w, D) / AV
    f8 = ml_dtypes.float8_e4m3
    wqb8 = np.clip(Wqbig * 16384.0, -240, 240).astype(f8)
    return dict(wqb8=wqb8, bqbig=(bqbig * AQ).astype(np.float32),
                wk=(Wk * AK).astype(bf), wv=(Wv * AV).astype(bf),
                wbig=(Wbig / (AV * SE)).astype(bf),
                wproj=w_proj.astype(bf), bwexp=bwexp.astype(np.float32),
                cbias=c_bias.astype(np.float32))


def kernel(**inputs):
    x = np.asarray(inputs["x"], np.float32)
    f = _fold(*[np.asarray(inputs[k], np.float32) for k in
                ("w_qkv", "b_qkv", "w_l", "w_w", "b_w", "w_proj", "b_proj")])

    if "nc" not in _CACHE:
        _CACHE["nc"] = build()
    nc = _CACHE["nc"]

    bf = ml_dtypes.bfloat16
    in_maps = []
    for core in range(8):
        b, half = core // 2, core % 2
        xT = np.ascontiguousarray(x[b].T)
        xh8 = np.clip(xT[:, half * NH:(half + 1) * NH] * 4.0,
                      -240, 240).astype(ml_dtypes.float8_e4m3)
        in_maps.append({
            "xt": xT.astype(bf),
            "xh8": np.ascontiguousarray(xh8),
            **f,
        })
    import os
    trace = bool(int(os.environ.get("BASSK_TRACE", "0")))
    res = run_bass_kernel_spmd(nc, in_maps, core_ids=list(range(8)),
                               trace=trace)
    _CACHE["last_results"] = res

    out = np.empty((B, N, DIM), np.float32)
    for core in range(8):
        b, half = core // 2, core % 2
        out[b, half * NH:(half + 1) * NH, :] = res.results[core]["out"].T
    return out
